# revision 9
# baseline (speedup 1.0000x reference)
"""Trainium2 Bass kernel for nn_Estor_raw_45595372814583.

Reference computation (B=64, L=512, H=768, I=3072, T=50, NL=9, S=4096):
    taged[b, s:e, :] += tag_emb[tag]      for each span (b, tag, s, e)
    x   = LN(word_embedding + 0.5 * taged) * g_att + be_att
    h   = relu(x @ w1 + b1) @ w2 + b2 + x
    h   = LN(h) * g_ff + be_ff
    out = h @ w_out + b_out               # [B, L, 9]

Strategy: data-parallel over batch across 8 cores (8 batches each). The
span scatter is computed on-device as two small matmuls per batch:
    in_span[s, l] = (l >= start_s) & (l < end_s)        (DVE compares vs iota)
    onehot[s, t]  = (tag_s == t)
    countT[t, l]  = onehot.T @ in_span                  (PE)
    taged[l, :]   = countT[:, l].T @ (0.5 * tag_emb)    (PE)
The FFN runs in fp16 on the PE with fp32 PSUM accumulation; LayerNorm
stats use bn_stats/bn_aggr on the DVE in fp32. Activation transposes go
through the (otherwise idle) DMA XBAR. Batches are software-pipelined:
while batch b runs its second FFN matmul, batch b+1's scatter+LN1 chain
executes on the DVE, and each chunk's output stage is delayed by one
chunk so its LN2 latency hides under the next chunk's matmuls.
"""

import math
import os
import sys

import numpy as np

for _p in ("/opt/trn_rl_repo", "/opt/trn_rl_repo/concourse"):
    if _p not in sys.path and os.path.isdir(_p):
        sys.path.insert(0, _p)

import concourse.bass as bass
import concourse.mybir as mybir
import concourse.tile as tile
from concourse.bass_utils import run_bass_kernel_spmd

B, L, H, I, T, NL = 64, 512, 768, 3072, 50, 9
RATE = 0.5
EPS = 1e-12
P = 128
N_CORES = 8
BPC = B // N_CORES          # batches per core
TOK = BPC * L               # tokens per core
KSUB = H // P               # 6   k-subtiles over H
ISUB = I // P               # 24  subtiles over I
NH = H // 2                 # 384 n-half for H-wide psum outputs

f32 = mybir.dt.float32
f16 = mybir.dt.float16


def build_kernel(nt_span: int, use_b1: bool, use_b2: bool, use_bout: bool,
                 use_gb_att: bool, use_gb_ff: bool):
    """Build the SPMD Bass program (same program on all 8 cores).

    nt_span: number of 128-span tiles per batch (spans padded to nt_span*128).
    """
    nc = bass.Bass()

    we = nc.declare_dram_parameter("we", [TOK, H], f32, isOutput=False)
    w1 = nc.declare_dram_parameter("w1", [H, I], f16, isOutput=False)
    w2 = nc.declare_dram_parameter("w2", [I, H], f16, isOutput=False)
    wout = nc.declare_dram_parameter("wout", [H, NL], f16, isOutput=False)
    temb = nc.declare_dram_parameter("temb", [T, H], f16, isOutput=False)
    spans = nc.declare_dram_parameter("spans", [BPC, nt_span, P, 3], f32, isOutput=False)
    iota_l = nc.declare_dram_parameter("iota_l", [L], f32, isOutput=False)
    iota_t = nc.declare_dram_parameter("iota_t", [P], f32, isOutput=False)
    b1 = nc.declare_dram_parameter("b1", [I], f32, isOutput=False) if use_b1 else None
    b2 = nc.declare_dram_parameter("b2", [H], f32, isOutput=False) if use_b2 else None
    bout = nc.declare_dram_parameter("bout", [NL], f32, isOutput=False) if use_bout else None
    gb_att = nc.declare_dram_parameter("gb_att", [2, H], f32, isOutput=False) if use_gb_att else None
    gb_ff = nc.declare_dram_parameter("gb_ff", [2, H], f32, isOutput=False) if use_gb_ff else None

    out = nc.declare_dram_parameter("out", [TOK, NL], f32, isOutput=True)

    from contextlib import ExitStack
    with tile.TileContext(nc) as tc, ExitStack() as ctx:
        const = ctx.enter_context(tc.tile_pool(name="const", bufs=1))
        wpool = ctx.enter_context(tc.tile_pool(name="weights", bufs=1))
        span_sb = ctx.enter_context(tc.tile_pool(name="span_sb", bufs=2))
        masks = ctx.enter_context(tc.tile_pool(name="masks", bufs=2))
        chunks = ctx.enter_context(tc.tile_pool(name="chunks", bufs=3))
        mega = ctx.enter_context(tc.tile_pool(name="mega", bufs=2))
        xtp = ctx.enter_context(tc.tile_pool(name="xtp", bufs=2))
        h1pool = ctx.enter_context(tc.tile_pool(name="h1pool", bufs=1))
        ctpool = ctx.enter_context(tc.tile_pool(name="ctpool", bufs=2))
        stats = ctx.enter_context(tc.tile_pool(name="stats", bufs=4))
        outsb = ctx.enter_context(tc.tile_pool(name="outsb", bufs=3))

        # PSUM budget (8 banks): half_a 2 + half_b 2 + ps1 2 + cnt 1 + ps3 1
        pp_cnt = ctx.enter_context(tc.tile_pool(name="pp_cnt", bufs=1, space="PSUM"))
        pp_out = ctx.enter_context(tc.tile_pool(name="pp_out", bufs=1, space="PSUM"))
        pp_mm1 = ctx.enter_context(tc.tile_pool(name="pp_mm1", bufs=2, space="PSUM"))
        pp_half = ctx.enter_context(tc.tile_pool(name="pp_half", bufs=2, space="PSUM"))

        # ---- persistent constants / weights ----
        eps_t = const.tile([P, 1], f32)
        nc.vector.memset(eps_t, EPS)
        iota_l_sb = const.tile([P, L], f32)
        nc.gpsimd.dma_start(out=iota_l_sb, in_=iota_l[None, :].to_broadcast([P, L]))
        iota_t_sb = const.tile([P, P], f32)
        nc.gpsimd.dma_start(out=iota_t_sb, in_=iota_t[None, :].to_broadcast([P, P]))

        w1_sb = wpool.tile([P, KSUB, I], f16)
        nc.sync.dma_start(out=w1_sb, in_=w1.rearrange("(s p) i -> p s i", p=P))
        w2_sb = wpool.tile([P, ISUB, H], f16)
        nc.sync.dma_start(out=w2_sb, in_=w2.rearrange("(s p) h -> p s h", p=P))
        wout_sb = wpool.tile([P, KSUB, NL], f16)
        nc.sync.dma_start(out=wout_sb, in_=wout.rearrange("(s p) n -> p s n", p=P))
        temb_sb = wpool.tile([P, H], f16)
        if T < P:
            nc.vector.memset(temb_sb, 0.0)
        nc.sync.dma_start(out=temb_sb[:T, :], in_=temb[:, :])

        b1_sb = None
        if b1 is not None:
            b1_sb = wpool.tile([P, ISUB], f32)
            nc.sync.dma_start(out=b1_sb, in_=b1.rearrange("(s p) -> p s", p=P))
        b2_sb = None
        if b2 is not None:
            b2_sb = wpool.tile([P, H], f32)
            nc.gpsimd.dma_start(out=b2_sb, in_=b2[None, :].to_broadcast([P, H]))
        bout_sb = None
        if bout is not None:
            bout_sb = wpool.tile([P, NL], f32)
            nc.gpsimd.dma_start(out=bout_sb, in_=bout[None, :].to_broadcast([P, NL]))
        gb_att_sb = None
        if gb_att is not None:
            gb_att_sb = wpool.tile([P, 2, H], f32)
            nc.gpsimd.dma_start(out=gb_att_sb, in_=gb_att[None, :, :].to_broadcast([P, 2, H]))
        gb_ff_sb = None
        if gb_ff is not None:
            gb_ff_sb = wpool.tile([P, 2, H], f32)
            nc.gpsimd.dma_start(out=gb_ff_sb, in_=gb_ff[None, :, :].to_broadcast([P, 2, H]))

        # rotating per-batch state (allocated by the prep stage)
        state = {}

        def emit_count(bp):
            """Span masks + count matmul -> countT_sb [128, L] f16 for batch bp."""
            cnt_psum = pp_cnt.tile([P, L], f32, tag="cnt")
            for st in range(nt_span):
                sp_t = span_sb.tile([P, 3], f32, tag="spans")
                nc.sync.dma_start(out=sp_t, in_=spans[bp, st, :, :])
                s_t, e_t, g_t = sp_t[:, 0:1], sp_t[:, 1:2], sp_t[:, 2:3]
                ge = masks.tile([P, L], f32, tag="ge")
                nc.vector.tensor_tensor(
                    out=ge, in0=iota_l_sb, in1=s_t.to_broadcast([P, L]),
                    op=mybir.AluOpType.is_ge)
                lt = masks.tile([P, L], f32, tag="lt")
                nc.vector.tensor_tensor(
                    out=lt, in0=iota_l_sb, in1=e_t.to_broadcast([P, L]),
                    op=mybir.AluOpType.is_lt)
                in_span = masks.tile([P, L], f16, tag="in_span")
                nc.vector.tensor_tensor(
                    out=in_span, in0=ge, in1=lt, op=mybir.AluOpType.mult)
                onehot = masks.tile([P, P], f16, tag="onehot")
                nc.vector.tensor_tensor(
                    out=onehot, in0=iota_t_sb, in1=g_t.to_broadcast([P, P]),
                    op=mybir.AluOpType.is_equal)
                nc.tensor.matmul(cnt_psum, lhsT=onehot, rhs=in_span,
                                 start=(st == 0), stop=(st == nt_span - 1))
            countT = ctpool.tile([P, L], f16, tag="countT")
            nc.vector.tensor_copy(out=countT, in_=cnt_psum)
            state[bp] = {"countT": countT}

        def emit_prep(bp, ci):
            """taged + LN1 + transpose for chunk ci of batch bp."""
            st_b = state[bp]
            if ci == 0:
                st_b["xn_f32"] = mega.tile([P, 4, H], f32, tag="xn_f32", name="xn_f32")
                st_b["xT"] = xtp.tile([P, KSUB, L], f16, tag="xT", name="xT")
            row0 = bp * L + ci * P
            tg_a = pp_half.tile([P, NH], f32, tag="half_a")
            tg_b = pp_half.tile([P, NH], f32, tag="half_b")
            csl = st_b["countT"][:, ci * P:(ci + 1) * P]
            nc.tensor.matmul(tg_a, lhsT=csl, rhs=temb_sb[:, :NH],
                             start=True, stop=True)
            nc.tensor.matmul(tg_b, lhsT=csl, rhs=temb_sb[:, NH:],
                             start=True, stop=True)
            we_t = chunks.tile([P, H], f32, tag="we")
            nc.sync.dma_start(out=we_t, in_=we[row0:row0 + P, :])
            xpre = chunks.tile([P, H], f32, tag="xpre")
            nc.vector.tensor_add(out=xpre[:, :NH], in0=we_t[:, :NH], in1=tg_a)
            nc.vector.tensor_add(out=xpre[:, NH:], in0=we_t[:, NH:], in1=tg_b)

            mean, rstd = _ln_stats(nc, stats, xpre, eps_t)
            xn32 = st_b["xn_f32"][:, ci, :]
            nc.vector.tensor_scalar(
                out=xn32, in0=xpre, scalar1=mean, scalar2=rstd,
                op0=mybir.AluOpType.subtract, op1=mybir.AluOpType.mult)
            if gb_att_sb is not None:
                nc.vector.tensor_mul(out=xn32, in0=xn32, in1=gb_att_sb[:, 0, :])
                nc.vector.tensor_add(out=xn32, in0=xn32, in1=gb_att_sb[:, 1, :])
            xn16 = chunks.tile([P, H], f16, tag="xn16")
            nc.vector.tensor_copy(out=xn16, in_=xn32)
            xT = st_b["xT"]
            for k in range(KSUB):
                nc.sync.dma_start_transpose(
                    xT[:, k, ci * P:(ci + 1) * P], xn16[:, k * P:(k + 1) * P])

        def emit_mm1(b):
            st_b = state[b]
            h1T = h1pool.tile([P, ISUB, L], f16, tag="h1T")
            st_b["h1T"] = h1T
            xT = st_b["xT"]
            for isub in range(ISUB):
                ps1 = pp_mm1.tile([P, L], f32, tag="ps1")
                for k in range(KSUB):
                    nc.tensor.matmul(
                        ps1, lhsT=w1_sb[:, k, isub * P:(isub + 1) * P],
                        rhs=xT[:, k, :], start=(k == 0), stop=(k == KSUB - 1))
                if b1_sb is not None:
                    nc.vector.tensor_scalar(
                        out=h1T[:, isub, :], in0=ps1,
                        scalar1=b1_sb[:, isub:isub + 1], scalar2=0.0,
                        op0=mybir.AluOpType.add, op1=mybir.AluOpType.max)
                else:
                    nc.vector.tensor_scalar(
                        out=h1T[:, isub, :], in0=ps1, scalar1=0.0, scalar2=None,
                        op0=mybir.AluOpType.max)
                if isub == 1 and b + 1 < BPC:
                    emit_count(b + 1)

        def emit_mm2(b, ci):
            st_b = state[b]
            h1T = st_b["h1T"]
            ps2a = pp_half.tile([P, NH], f32, tag="half_a")
            ps2b = pp_half.tile([P, NH], f32, tag="half_b")
            for isub in range(ISUB):
                lhs = h1T[:, isub, ci * P:(ci + 1) * P]
                nc.tensor.matmul(ps2a, lhsT=lhs, rhs=w2_sb[:, isub, :NH],
                                 start=(isub == 0), stop=(isub == ISUB - 1))
                nc.tensor.matmul(ps2b, lhsT=lhs, rhs=w2_sb[:, isub, NH:],
                                 start=(isub == 0), stop=(isub == ISUB - 1))
            st_b[("ps2", ci)] = (ps2a, ps2b)

        def emit_out(b, ci):
            st_b = state[b]
            ps2a, ps2b = st_b.pop(("ps2", ci))
            xn32 = st_b["xn_f32"][:, ci, :]
            row0 = b * L + ci * P
            h2 = chunks.tile([P, H], f32, tag="h2")
            nc.vector.tensor_add(out=h2[:, :NH], in0=ps2a, in1=xn32[:, :NH])
            nc.vector.tensor_add(out=h2[:, NH:], in0=ps2b, in1=xn32[:, NH:])
            if b2_sb is not None:
                nc.vector.tensor_add(out=h2, in0=h2, in1=b2_sb)

            mean2, rstd2 = _ln_stats(nc, stats, h2, eps_t)
            h2n = chunks.tile([P, H], f16, tag="h2n")
            nc.vector.tensor_scalar(
                out=h2n, in0=h2, scalar1=mean2, scalar2=rstd2,
                op0=mybir.AluOpType.subtract, op1=mybir.AluOpType.mult)
            if gb_ff_sb is not None:
                nc.vector.tensor_mul(out=h2n, in0=h2n, in1=gb_ff_sb[:, 0, :])
                nc.vector.tensor_add(out=h2n, in0=h2n, in1=gb_ff_sb[:, 1, :])

            h2nT = chunks.tile([P, KSUB, P], f16, tag="h2nT")
            for k in range(KSUB):
                nc.sync.dma_start_transpose(
                    h2nT[:, k, :], h2n[:, k * P:(k + 1) * P])
            ps3 = pp_out.tile([P, NL], f32, tag="ps3")
            for k in range(KSUB):
                nc.tensor.matmul(ps3, lhsT=h2nT[:, k, :], rhs=wout_sb[:, k, :],
                                 start=(k == 0), stop=(k == KSUB - 1))
            o_t = outsb.tile([P, NL], f32, tag="o")
            if bout_sb is not None:
                nc.vector.tensor_add(out=o_t, in0=ps3, in1=bout_sb)
            else:
                nc.vector.tensor_copy(out=o_t, in_=ps3)
            nc.sync.dma_start(out=out[row0:row0 + P, :], in_=o_t)

        # ---- pipelined emission ----
        emit_count(0)
        for ci in range(4):
            emit_prep(0, ci)
        pending = []
        for b in range(BPC):
            emit_mm1(b)
            for ci in range(4):
                if pending:
                    emit_out(*pending.pop(0))
                if b + 1 < BPC:
                    emit_prep(b + 1, ci)
                emit_mm2(b, ci)
                pending.append((b, ci))
            if b > 0:
                del state[b - 1]
        while pending:
            emit_out(*pending.pop(0))

    _split_multi_waits(nc)
    return nc


def _ln_stats(nc, stats_pool, x, eps_t):
    """mean/rstd over the free dim (H=768) via bn_stats in 256-wide groups."""
    sub = 256
    n_sub = H // sub
    st = stats_pool.tile([P, n_sub, 6], f32, tag="bn_st")
    xg = x.rearrange("p (n s) -> p n s", s=sub)
    for i in range(n_sub):
        nc.vector.bn_stats(out=st[:, i, :], in_=xg[:, i, :])
    mv = stats_pool.tile([P, 2], f32, tag="bn_mv")
    nc.vector.bn_aggr(out=mv, in_=st)
    rstd = stats_pool.tile([P, 1], f32, tag="rstd")
    nc.scalar.activation(out=rstd, in_=mv[:, 1:2],
                         func=mybir.ActivationFunctionType.Sqrt,
                         bias=eps_t, scale=1.0)
    nc.vector.reciprocal(out=rstd, in_=rstd)
    return mv[:, 0:1], rstd


def _split_multi_waits(nc, max_waits=1):
    """walrus codegen in this toolchain accepts at most one sync wait per
    compute instruction; hoist extras onto same-engine NoOps just before."""
    n_nops = 0
    for f in nc.m.functions:
        for blk in f.blocks:
            insts = blk.instructions
            out = []
            changed = False
            for inst in insts:
                si = getattr(inst, "sync_info", None)
                waits = list(si.on_wait) if si is not None and si.on_wait else []
                if len(waits) > max_waits:
                    for w in waits[:-max_waits]:
                        nop = mybir.InstNoOp(
                            name=f"W-split-{n_nops}", ins=[], outs=[])
                        nop.engine = inst.engine
                        nop.sync_info = mybir.SyncInfo(on_wait=[w], on_update=[])
                        out.append(nop)
                        n_nops += 1
                    inst.sync_info = mybir.SyncInfo(
                        on_wait=waits[-max_waits:], on_update=list(si.on_update))
                    changed = True
                out.append(inst)
            if changed:
                blk.instructions = out
    return n_nops


_BUILT = {}


def _prep_inputs(word_embedding, tag_emb, w1, b1, w2, b2, g_att, be_att,
                 g_ff, be_ff, w_out, b_out, span_b, span_tag, span_start,
                 span_end):
    """Host-side sharding: bucket spans by batch, cast weights, build in_maps."""
    we = np.ascontiguousarray(np.asarray(word_embedding, np.float32))
    sb = np.asarray(span_b).astype(np.int64)
    stg = np.asarray(span_tag).astype(np.int64)
    ss = np.asarray(span_start).astype(np.int64)
    se = np.asarray(span_end).astype(np.int64)

    counts = np.bincount(sb, minlength=B)
    nt_span = max(1, math.ceil(counts.max() / P))
    smax = nt_span * P
    spans = np.zeros((B, smax, 3), np.float32)
    spans[:, :, 2] = -1.0  # tag -1 never matches iota_t
    for b in range(B):
        idx = np.flatnonzero(sb == b)
        n = len(idx)
        spans[b, :n, 0] = ss[idx]
        spans[b, :n, 1] = se[idx]
        spans[b, :n, 2] = stg[idx]

    w1h = np.asarray(w1, np.float32).astype(np.float16)
    w2h = np.asarray(w2, np.float32).astype(np.float16)
    wouth = np.asarray(w_out, np.float32).astype(np.float16)
    tembh = (np.asarray(tag_emb, np.float32) * RATE).astype(np.float16)

    b1_ = np.asarray(b1, np.float32)
    b2_ = np.asarray(b2, np.float32)
    bout_ = np.asarray(b_out, np.float32)
    ga = np.asarray(g_att, np.float32)
    ba = np.asarray(be_att, np.float32)
    gf = np.asarray(g_ff, np.float32)
    bf = np.asarray(be_ff, np.float32)
    use_b1 = bool(np.any(b1_ != 0))
    use_b2 = bool(np.any(b2_ != 0))
    use_bout = bool(np.any(bout_ != 0))
    use_gb_att = bool(np.any(ga != 1) or np.any(ba != 0))
    use_gb_ff = bool(np.any(gf != 1) or np.any(bf != 0))

    iota_l = np.arange(L, dtype=np.float32)
    iota_t = np.arange(P, dtype=np.float32)

    in_maps = []
    for c in range(N_CORES):
        b0 = c * BPC
        m = dict(
            we=we[b0:b0 + BPC].reshape(TOK, H),
            w1=w1h, w2=w2h, wout=wouth, temb=tembh,
            spans=spans[b0:b0 + BPC].reshape(BPC, nt_span, P, 3),
            iota_l=iota_l, iota_t=iota_t,
        )
        if use_b1:
            m["b1"] = b1_
        if use_b2:
            m["b2"] = b2_
        if use_bout:
            m["bout"] = bout_
        if use_gb_att:
            m["gb_att"] = np.stack([ga, ba])
        if use_gb_ff:
            m["gb_ff"] = np.stack([gf, bf])
        in_maps.append(m)

    key = (nt_span, use_b1, use_b2, use_bout, use_gb_att, use_gb_ff)
    return key, in_maps


def kernel(**inputs):
    key, in_maps = _prep_inputs(**inputs)
    if key not in _BUILT:
        _BUILT[key] = build_kernel(*key)
    nc = _BUILT[key]
    res = run_bass_kernel_spmd(nc, in_maps, core_ids=list(range(N_CORES)))
    outs = [res.results[c]["out"].reshape(BPC, L, NL) for c in range(N_CORES)]
    return np.concatenate(outs, axis=0).astype(np.float32)


# revision 11
# speedup vs baseline: 1.5868x; 1.5868x over previous
"""Trainium2 Bass kernel for nn_Estor_raw_45595372814583.

Reference computation (B=64, L=512, H=768, I=3072, T=50, NL=9, S=4096):
    taged[b, s:e, :] += tag_emb[tag]      for each span (b, tag, s, e)
    x   = LN(word_embedding + 0.5 * taged) * g_att + be_att
    h   = relu(x @ w1 + b1) @ w2 + b2 + x
    h   = LN(h) * g_ff + be_ff
    out = h @ w_out + b_out               # [B, L, 9]

Strategy: data-parallel over batch across 8 cores (8 batches each). The
span scatter is computed on-device as two small matmuls per batch:
    in_span[s, l] = (l >= start_s) & (l < end_s)        (DVE compares vs iota)
    onehot[s, t]  = (tag_s == t)
    countT[t, l]  = onehot.T @ in_span                  (PE)
    taged[l, :]   = countT[:, l].T @ (0.5 * tag_emb)    (PE)
The FFN runs in fp16 on the PE with fp32 PSUM accumulation; LayerNorm
stats use bn_stats/bn_aggr on the DVE in fp32. Activation transposes go
through the (otherwise idle) DMA XBAR. Batches are software-pipelined:
while batch b runs its second FFN matmul, batch b+1's scatter+LN1 chain
executes on the DVE, and each chunk's output stage is delayed by one
chunk so its LN2 latency hides under the next chunk's matmuls.
"""

import math
import os
import sys

import numpy as np

for _p in ("/opt/trn_rl_repo", "/opt/trn_rl_repo/concourse"):
    if _p not in sys.path and os.path.isdir(_p):
        sys.path.insert(0, _p)

import concourse.bass as bass
import concourse.mybir as mybir
import concourse.tile as tile
from concourse.bass_utils import run_bass_kernel_spmd
from concourse.masks import make_identity

B, L, H, I, T, NL = 64, 512, 768, 3072, 50, 9
RATE = 0.5
EPS = 1e-12
P = 128
N_CORES = 8
BPC = B // N_CORES          # batches per core
TOK = BPC * L               # tokens per core
KSUB = H // P               # 6   k-subtiles over H
ISUB = I // P               # 24  subtiles over I
NH = H // 2                 # 384 n-half for H-wide psum outputs

f32 = mybir.dt.float32
f16 = mybir.dt.float16


def build_kernel(nt_span: int, use_b1: bool, use_b2: bool, use_bout: bool,
                 use_gb_att: bool, use_gb_ff: bool):
    """Build the SPMD Bass program (same program on all 8 cores).

    nt_span: number of 128-span tiles per batch (spans padded to nt_span*128).
    """
    nc = bass.Bass()

    we = nc.declare_dram_parameter("we", [TOK, H], f32, isOutput=False)
    w1 = nc.declare_dram_parameter("w1", [H, I], f16, isOutput=False)
    w2 = nc.declare_dram_parameter("w2", [I, H], f16, isOutput=False)
    wout = nc.declare_dram_parameter("wout", [H, NL], f16, isOutput=False)
    temb = nc.declare_dram_parameter("temb", [T, H], f16, isOutput=False)
    spans = nc.declare_dram_parameter("spans", [BPC, nt_span, P, 3], f32, isOutput=False)
    iota_l = nc.declare_dram_parameter("iota_l", [L], f32, isOutput=False)
    iota_t = nc.declare_dram_parameter("iota_t", [P], f32, isOutput=False)
    b1 = nc.declare_dram_parameter("b1", [I], f32, isOutput=False) if use_b1 else None
    b2 = nc.declare_dram_parameter("b2", [H], f32, isOutput=False) if use_b2 else None
    bout = nc.declare_dram_parameter("bout", [NL], f32, isOutput=False) if use_bout else None
    gb_att = nc.declare_dram_parameter("gb_att", [2, H], f32, isOutput=False) if use_gb_att else None
    gb_ff = nc.declare_dram_parameter("gb_ff", [2, H], f32, isOutput=False) if use_gb_ff else None

    out = nc.declare_dram_parameter("out", [TOK, NL], f32, isOutput=True)

    from contextlib import ExitStack
    with tile.TileContext(nc) as tc, ExitStack() as ctx:
        const = ctx.enter_context(tc.tile_pool(name="const", bufs=1))
        wpool = ctx.enter_context(tc.tile_pool(name="weights", bufs=1))
        span_sb = ctx.enter_context(tc.tile_pool(name="span_sb", bufs=2))
        masks = ctx.enter_context(tc.tile_pool(name="masks", bufs=2))
        chunks = ctx.enter_context(tc.tile_pool(name="chunks", bufs=3))
        mega = ctx.enter_context(tc.tile_pool(name="mega", bufs=2))
        xtp = ctx.enter_context(tc.tile_pool(name="xtp", bufs=2))
        h1pool = ctx.enter_context(tc.tile_pool(name="h1pool", bufs=1))
        ctpool = ctx.enter_context(tc.tile_pool(name="ctpool", bufs=2))
        stats = ctx.enter_context(tc.tile_pool(name="stats", bufs=4))
        outsb = ctx.enter_context(tc.tile_pool(name="outsb", bufs=3))

        # PSUM budget (8 banks): aux 4 + ps2 2 + ps1 2
        pp_aux = ctx.enter_context(tc.tile_pool(name="pp_aux", bufs=4, space="PSUM"))
        pp_mm1 = ctx.enter_context(tc.tile_pool(name="pp_mm1", bufs=2, space="PSUM"))
        pp_mm2 = ctx.enter_context(tc.tile_pool(name="pp_mm2", bufs=2, space="PSUM"))

        # ---- persistent constants / weights ----
        ident = const.tile([P, P], f16)
        make_identity(nc, ident)
        eps_t = const.tile([P, 1], f32)
        nc.vector.memset(eps_t, EPS)
        iota_l_sb = const.tile([P, L], f32)
        nc.gpsimd.dma_start(out=iota_l_sb, in_=iota_l[None, :].to_broadcast([P, L]))
        iota_t_sb = const.tile([P, P], f32)
        nc.gpsimd.dma_start(out=iota_t_sb, in_=iota_t[None, :].to_broadcast([P, P]))

        w1_sb = wpool.tile([P, KSUB, I], f16)
        nc.sync.dma_start(out=w1_sb, in_=w1.rearrange("(s p) i -> p s i", p=P))
        w2_sb = wpool.tile([P, ISUB, H], f16)
        nc.sync.dma_start(out=w2_sb, in_=w2.rearrange("(s p) h -> p s h", p=P))
        wout_sb = wpool.tile([P, KSUB, NL], f16)
        nc.sync.dma_start(out=wout_sb, in_=wout.rearrange("(s p) n -> p s n", p=P))
        temb_sb = wpool.tile([P, H], f16)
        if T < P:
            nc.vector.memset(temb_sb, 0.0)
        nc.sync.dma_start(out=temb_sb[:T, :], in_=temb[:, :])

        b1_sb = None
        if b1 is not None:
            b1_sb = wpool.tile([P, ISUB], f32)
            nc.sync.dma_start(out=b1_sb, in_=b1.rearrange("(s p) -> p s", p=P))
        b2_sb = None
        if b2 is not None:
            b2_sb = wpool.tile([P, H], f32)
            nc.gpsimd.dma_start(out=b2_sb, in_=b2[None, :].to_broadcast([P, H]))
        bout_sb = None
        if bout is not None:
            bout_sb = wpool.tile([P, NL], f32)
            nc.gpsimd.dma_start(out=bout_sb, in_=bout[None, :].to_broadcast([P, NL]))
        gb_att_sb = None
        if gb_att is not None:
            gb_att_sb = wpool.tile([P, 2, H], f32)
            nc.gpsimd.dma_start(out=gb_att_sb, in_=gb_att[None, :, :].to_broadcast([P, 2, H]))
        gb_ff_sb = None
        if gb_ff is not None:
            gb_ff_sb = wpool.tile([P, 2, H], f32)
            nc.gpsimd.dma_start(out=gb_ff_sb, in_=gb_ff[None, :, :].to_broadcast([P, 2, H]))

        # rotating per-batch state (allocated by the prep stage)
        state = {}

        def emit_count(bp):
            """Span masks + count matmul -> countT_sb [128, L] f16 for batch bp."""
            cnt_psum = pp_aux.tile([P, L], f32, tag="aux")
            for st in range(nt_span):
                sp_t = span_sb.tile([P, 3], f32, tag="spans")
                nc.sync.dma_start(out=sp_t, in_=spans[bp, st, :, :])
                s_t, e_t, g_t = sp_t[:, 0:1], sp_t[:, 1:2], sp_t[:, 2:3]
                ge = masks.tile([P, L], f32, tag="ge")
                nc.vector.tensor_tensor(
                    out=ge, in0=iota_l_sb, in1=s_t.to_broadcast([P, L]),
                    op=mybir.AluOpType.is_ge)
                lt = masks.tile([P, L], f32, tag="lt")
                nc.vector.tensor_tensor(
                    out=lt, in0=iota_l_sb, in1=e_t.to_broadcast([P, L]),
                    op=mybir.AluOpType.is_lt)
                in_span = masks.tile([P, L], f16, tag="in_span")
                nc.vector.tensor_tensor(
                    out=in_span, in0=ge, in1=lt, op=mybir.AluOpType.mult)
                onehot = masks.tile([P, P], f16, tag="onehot")
                nc.vector.tensor_tensor(
                    out=onehot, in0=iota_t_sb, in1=g_t.to_broadcast([P, P]),
                    op=mybir.AluOpType.is_equal)
                nc.tensor.matmul(cnt_psum, lhsT=onehot, rhs=in_span,
                                 start=(st == 0), stop=(st == nt_span - 1))
            countT = ctpool.tile([P, L], f16, tag="countT")
            nc.vector.tensor_copy(out=countT, in_=cnt_psum)
            state[bp] = {"countT": countT}

        def prep_head(bp, ci):
            """taged + LN1 chain (PE: 2 small matmuls; rest DVE)."""
            st_b = state[bp]
            if ci == 0:
                st_b["xn_f32"] = mega.tile([P, 4, H], f32, tag="xn_f32", name="xn_f32")
                st_b["xT"] = xtp.tile([P, KSUB, L], f16, tag="xT", name="xT")
            row0 = bp * L + ci * P
            tg_a = pp_aux.tile([P, NH], f32, tag="aux", name="tg_a")
            tg_b = pp_aux.tile([P, NH], f32, tag="aux", name="tg_b")
            csl = st_b["countT"][:, ci * P:(ci + 1) * P]
            nc.tensor.matmul(tg_a, lhsT=csl, rhs=temb_sb[:, :NH],
                             start=True, stop=True)
            nc.tensor.matmul(tg_b, lhsT=csl, rhs=temb_sb[:, NH:],
                             start=True, stop=True)
            we_t = chunks.tile([P, H], f32, tag="we")
            nc.sync.dma_start(out=we_t, in_=we[row0:row0 + P, :])
            xpre = chunks.tile([P, H], f32, tag="xpre")
            nc.vector.tensor_add(out=xpre[:, :NH], in0=we_t[:, :NH], in1=tg_a)
            nc.vector.tensor_add(out=xpre[:, NH:], in0=we_t[:, NH:], in1=tg_b)

            mean, rstd = _ln_stats(nc, stats, xpre, eps_t)
            xn32 = st_b["xn_f32"][:, ci, :]
            nc.vector.tensor_scalar(
                out=xn32, in0=xpre, scalar1=mean, scalar2=rstd,
                op0=mybir.AluOpType.subtract, op1=mybir.AluOpType.mult)
            if gb_att_sb is not None:
                nc.vector.tensor_mul(out=xn32, in0=xn32, in1=gb_att_sb[:, 0, :])
                nc.vector.tensor_add(out=xn32, in0=xn32, in1=gb_att_sb[:, 1, :])
            xn16 = chunks.tile([P, H], f16, tag="xn16")
            nc.vector.tensor_copy(out=xn16, in_=xn32)
            st_b[("xn16", ci)] = xn16

        def prep_tail(bp, ci):
            """PE transposes of xn16 into xT (emitted after a matmul burst
            so the LN1 chain has already finished on the DVE)."""
            st_b = state[bp]
            xn16 = st_b.pop(("xn16", ci))
            tp_x = pp_aux.tile([P, H], f16, tag="aux", name="tp_x")
            for k in range(KSUB):
                nc.tensor.transpose(tp_x[:, k * P:(k + 1) * P],
                                    xn16[:, k * P:(k + 1) * P], ident)
            nc.vector.tensor_copy(
                out=st_b["xT"][:, :, ci * P:(ci + 1) * P],
                in_=tp_x.rearrange("p (k t) -> p k t", t=P))

        def emit_mm1(b):
            st_b = state[b]
            h1T = h1pool.tile([P, ISUB, L], f16, tag="h1T")
            st_b["h1T"] = h1T
            xT = st_b["xT"]
            for isub in range(ISUB):
                ps1 = pp_mm1.tile([P, L], f32, tag="ps1")
                for k in range(KSUB):
                    nc.tensor.matmul(
                        ps1, lhsT=w1_sb[:, k, isub * P:(isub + 1) * P],
                        rhs=xT[:, k, :], start=(k == 0), stop=(k == KSUB - 1))
                if b1_sb is not None:
                    nc.vector.tensor_scalar(
                        out=h1T[:, isub, :], in0=ps1,
                        scalar1=b1_sb[:, isub:isub + 1], scalar2=0.0,
                        op0=mybir.AluOpType.add, op1=mybir.AluOpType.max)
                else:
                    nc.vector.tensor_scalar(
                        out=h1T[:, isub, :], in0=ps1, scalar1=0.0, scalar2=None,
                        op0=mybir.AluOpType.max)
                if isub == 1 and b + 1 < BPC:
                    emit_count(b + 1)

        def emit_mm2(b, ci):
            st_b = state[b]
            h1T = st_b["h1T"]
            ps2a = pp_mm2.tile([P, NH], f32, tag="ps2", name="ps2a")
            for isub in range(ISUB):
                nc.tensor.matmul(ps2a, lhsT=h1T[:, isub, ci * P:(ci + 1) * P],
                                 rhs=w2_sb[:, isub, :NH],
                                 start=(isub == 0), stop=(isub == ISUB - 1))
            ps2b = pp_mm2.tile([P, NH], f32, tag="ps2", name="ps2b")
            for isub in range(ISUB):
                nc.tensor.matmul(ps2b, lhsT=h1T[:, isub, ci * P:(ci + 1) * P],
                                 rhs=w2_sb[:, isub, NH:],
                                 start=(isub == 0), stop=(isub == ISUB - 1))
            st_b[("ps2", ci)] = (ps2a, ps2b)

        def out_dve(b, ci):
            """residual + LN2 on the DVE (no PE work)."""
            st_b = state[b]
            ps2a, ps2b = st_b.pop(("ps2", ci))
            xn32 = st_b["xn_f32"][:, ci, :]
            h2 = chunks.tile([P, H], f32, tag="h2")
            nc.vector.tensor_add(out=h2[:, :NH], in0=ps2a, in1=xn32[:, :NH])
            nc.vector.tensor_add(out=h2[:, NH:], in0=ps2b, in1=xn32[:, NH:])
            if b2_sb is not None:
                nc.vector.tensor_add(out=h2, in0=h2, in1=b2_sb)

            mean2, rstd2 = _ln_stats(nc, stats, h2, eps_t)
            h2n = chunks.tile([P, H], f16, tag="h2n")
            nc.vector.tensor_scalar(
                out=h2n, in0=h2, scalar1=mean2, scalar2=rstd2,
                op0=mybir.AluOpType.subtract, op1=mybir.AluOpType.mult)
            if gb_ff_sb is not None:
                nc.vector.tensor_mul(out=h2n, in0=h2n, in1=gb_ff_sb[:, 0, :])
                nc.vector.tensor_add(out=h2n, in0=h2n, in1=gb_ff_sb[:, 1, :])
            st_b[("h2n", ci)] = h2n

        def out_pe(b, ci):
            """h2n transposes + output projection (emitted after a matmul
            burst so LN2 has already finished on the DVE)."""
            st_b = state[b]
            h2n = st_b.pop(("h2n", ci))
            row0 = b * L + ci * P
            tp_h = pp_aux.tile([P, H], f16, tag="aux", name="tp_h")
            for k in range(KSUB):
                nc.tensor.transpose(tp_h[:, k * P:(k + 1) * P],
                                    h2n[:, k * P:(k + 1) * P], ident)
            h2nT = chunks.tile([P, H], f16, tag="h2nT")
            nc.vector.tensor_copy(out=h2nT, in_=tp_h)
            ps3 = pp_aux.tile([P, NL], f32, tag="aux", name="ps3")
            for k in range(KSUB):
                nc.tensor.matmul(ps3, lhsT=h2nT[:, k * P:(k + 1) * P],
                                 rhs=wout_sb[:, k, :],
                                 start=(k == 0), stop=(k == KSUB - 1))
            o_t = outsb.tile([P, NL], f32, tag="o")
            if bout_sb is not None:
                nc.vector.tensor_add(out=o_t, in0=ps3, in1=bout_sb)
            else:
                nc.vector.tensor_copy(out=o_t, in_=ps3)
            nc.sync.dma_start(out=out[row0:row0 + P, :], in_=o_t)

        # ---- pipelined emission ----
        emit_count(0)
        for ci in range(4):
            prep_head(0, ci)
            prep_tail(0, ci)
        pending = []
        for b in range(BPC):
            emit_mm1(b)
            for ci in range(4):
                if pending:
                    out_dve(*pending[0])
                if b + 1 < BPC:
                    prep_head(b + 1, ci)
                emit_mm2(b, ci)
                if b + 1 < BPC:
                    prep_tail(b + 1, ci)
                if pending:
                    out_pe(*pending.pop(0))
                pending.append((b, ci))
            if b > 1:
                del state[b - 2]
        while pending:
            b, ci = pending.pop(0)
            out_dve(b, ci)
            out_pe(b, ci)

    _split_multi_waits(nc)
    return nc


def _ln_stats(nc, stats_pool, x, eps_t):
    """mean/rstd over the free dim (H=768) via bn_stats in 256-wide groups."""
    sub = 256
    n_sub = H // sub
    st = stats_pool.tile([P, n_sub, 6], f32, tag="bn_st")
    xg = x.rearrange("p (n s) -> p n s", s=sub)
    for i in range(n_sub):
        nc.vector.bn_stats(out=st[:, i, :], in_=xg[:, i, :])
    mv = stats_pool.tile([P, 2], f32, tag="bn_mv")
    nc.vector.bn_aggr(out=mv, in_=st)
    rstd = stats_pool.tile([P, 1], f32, tag="rstd")
    nc.scalar.activation(out=rstd, in_=mv[:, 1:2],
                         func=mybir.ActivationFunctionType.Sqrt,
                         bias=eps_t, scale=1.0)
    nc.vector.reciprocal(out=rstd, in_=rstd)
    return mv[:, 0:1], rstd


def _split_multi_waits(nc, max_waits=1):
    """walrus codegen in this toolchain accepts at most one sync wait per
    compute instruction; hoist extras onto same-engine NoOps just before."""
    n_nops = 0
    for f in nc.m.functions:
        for blk in f.blocks:
            insts = blk.instructions
            out = []
            changed = False
            for inst in insts:
                si = getattr(inst, "sync_info", None)
                waits = list(si.on_wait) if si is not None and si.on_wait else []
                if len(waits) > max_waits:
                    for w in waits[:-max_waits]:
                        nop = mybir.InstNoOp(
                            name=f"W-split-{n_nops}", ins=[], outs=[])
                        nop.engine = inst.engine
                        nop.sync_info = mybir.SyncInfo(on_wait=[w], on_update=[])
                        out.append(nop)
                        n_nops += 1
                    inst.sync_info = mybir.SyncInfo(
                        on_wait=waits[-max_waits:], on_update=list(si.on_update))
                    changed = True
                out.append(inst)
            if changed:
                blk.instructions = out
    return n_nops


_BUILT = {}


def _prep_inputs(word_embedding, tag_emb, w1, b1, w2, b2, g_att, be_att,
                 g_ff, be_ff, w_out, b_out, span_b, span_tag, span_start,
                 span_end):
    """Host-side sharding: bucket spans by batch, cast weights, build in_maps."""
    we = np.ascontiguousarray(np.asarray(word_embedding, np.float32))
    sb = np.asarray(span_b).astype(np.int64)
    stg = np.asarray(span_tag).astype(np.int64)
    ss = np.asarray(span_start).astype(np.int64)
    se = np.asarray(span_end).astype(np.int64)

    counts = np.bincount(sb, minlength=B)
    nt_span = max(1, math.ceil(counts.max() / P))
    smax = nt_span * P
    spans = np.zeros((B, smax, 3), np.float32)
    spans[:, :, 2] = -1.0  # tag -1 never matches iota_t
    for b in range(B):
        idx = np.flatnonzero(sb == b)
        n = len(idx)
        spans[b, :n, 0] = ss[idx]
        spans[b, :n, 1] = se[idx]
        spans[b, :n, 2] = stg[idx]

    w1h = np.asarray(w1, np.float32).astype(np.float16)
    w2h = np.asarray(w2, np.float32).astype(np.float16)
    wouth = np.asarray(w_out, np.float32).astype(np.float16)
    tembh = (np.asarray(tag_emb, np.float32) * RATE).astype(np.float16)

    b1_ = np.asarray(b1, np.float32)
    b2_ = np.asarray(b2, np.float32)
    bout_ = np.asarray(b_out, np.float32)
    ga = np.asarray(g_att, np.float32)
    ba = np.asarray(be_att, np.float32)
    gf = np.asarray(g_ff, np.float32)
    bf = np.asarray(be_ff, np.float32)
    use_b1 = bool(np.any(b1_ != 0))
    use_b2 = bool(np.any(b2_ != 0))
    use_bout = bool(np.any(bout_ != 0))
    use_gb_att = bool(np.any(ga != 1) or np.any(ba != 0))
    use_gb_ff = bool(np.any(gf != 1) or np.any(bf != 0))

    iota_l = np.arange(L, dtype=np.float32)
    iota_t = np.arange(P, dtype=np.float32)

    in_maps = []
    for c in range(N_CORES):
        b0 = c * BPC
        m = dict(
            we=we[b0:b0 + BPC].reshape(TOK, H),
            w1=w1h, w2=w2h, wout=wouth, temb=tembh,
            spans=spans[b0:b0 + BPC].reshape(BPC, nt_span, P, 3),
            iota_l=iota_l, iota_t=iota_t,
        )
        if use_b1:
            m["b1"] = b1_
        if use_b2:
            m["b2"] = b2_
        if use_bout:
            m["bout"] = bout_
        if use_gb_att:
            m["gb_att"] = np.stack([ga, ba])
        if use_gb_ff:
            m["gb_ff"] = np.stack([gf, bf])
        in_maps.append(m)

    key = (nt_span, use_b1, use_b2, use_bout, use_gb_att, use_gb_ff)
    return key, in_maps


def kernel(**inputs):
    key, in_maps = _prep_inputs(**inputs)
    if key not in _BUILT:
        _BUILT[key] = build_kernel(*key)
    nc = _BUILT[key]
    res = run_bass_kernel_spmd(nc, in_maps, core_ids=list(range(N_CORES)))
    outs = [res.results[c]["out"].reshape(BPC, L, NL) for c in range(N_CORES)]
    return np.concatenate(outs, axis=0).astype(np.float32)


# revision 12
# speedup vs baseline: 1.5956x; 1.0056x over previous
"""Trainium2 Bass kernel for nn_Estor_raw_45595372814583.

Reference computation (B=64, L=512, H=768, I=3072, T=50, NL=9, S=4096):
    taged[b, s:e, :] += tag_emb[tag]      for each span (b, tag, s, e)
    x   = LN(word_embedding + 0.5 * taged) * g_att + be_att
    h   = relu(x @ w1 + b1) @ w2 + b2 + x
    h   = LN(h) * g_ff + be_ff
    out = h @ w_out + b_out               # [B, L, 9]

Strategy: data-parallel over batch across 8 cores (8 batches each). The
span scatter is computed on-device as two small matmuls per batch:
    in_span[s, l] = (l >= start_s) & (l < end_s)        (DVE compares vs iota)
    onehot[s, t]  = (tag_s == t)
    countT[t, l]  = onehot.T @ in_span                  (PE)
    taged[l, :]   = countT[:, l].T @ (0.5 * tag_emb)    (PE)
The FFN runs in fp16 on the PE with fp32 PSUM accumulation; LayerNorm
stats use bn_stats/bn_aggr on the DVE in fp32. Activation transposes go
through the (otherwise idle) DMA XBAR. Batches are software-pipelined:
while batch b runs its second FFN matmul, batch b+1's scatter+LN1 chain
executes on the DVE, and each chunk's output stage is delayed by one
chunk so its LN2 latency hides under the next chunk's matmuls.
"""

import math
import os
import sys

import numpy as np

for _p in ("/opt/trn_rl_repo", "/opt/trn_rl_repo/concourse"):
    if _p not in sys.path and os.path.isdir(_p):
        sys.path.insert(0, _p)

import concourse.bass as bass
import concourse.mybir as mybir
import concourse.tile as tile
from concourse.bass_utils import run_bass_kernel_spmd
from concourse.masks import make_identity

B, L, H, I, T, NL = 64, 512, 768, 3072, 50, 9
RATE = 0.5
EPS = 1e-12
P = 128
N_CORES = 8
BPC = B // N_CORES          # batches per core
TOK = BPC * L               # tokens per core
KSUB = H // P               # 6   k-subtiles over H
ISUB = I // P               # 24  subtiles over I
NH = H // 2                 # 384 n-half for H-wide psum outputs

f32 = mybir.dt.float32
f16 = mybir.dt.float16


def build_kernel(nt_span: int, use_b1: bool, use_b2: bool, use_bout: bool,
                 use_gb_att: bool, use_gb_ff: bool):
    """Build the SPMD Bass program (same program on all 8 cores).

    nt_span: number of 128-span tiles per batch (spans padded to nt_span*128).
    """
    nc = bass.Bass()

    we = nc.declare_dram_parameter("we", [TOK, H], f32, isOutput=False)
    w1 = nc.declare_dram_parameter("w1", [H, I], f16, isOutput=False)
    w2 = nc.declare_dram_parameter("w2", [I, H], f16, isOutput=False)
    wout = nc.declare_dram_parameter("wout", [H, NL], f16, isOutput=False)
    temb = nc.declare_dram_parameter("temb", [T, H], f16, isOutput=False)
    spans = nc.declare_dram_parameter("spans", [BPC, nt_span, P, 3], f32, isOutput=False)
    iota_l = nc.declare_dram_parameter("iota_l", [L], f32, isOutput=False)
    iota_t = nc.declare_dram_parameter("iota_t", [P], f32, isOutput=False)
    b1 = nc.declare_dram_parameter("b1", [I], f32, isOutput=False) if use_b1 else None
    b2 = nc.declare_dram_parameter("b2", [H], f32, isOutput=False) if use_b2 else None
    bout = nc.declare_dram_parameter("bout", [NL], f32, isOutput=False) if use_bout else None
    gb_att = nc.declare_dram_parameter("gb_att", [2, H], f32, isOutput=False) if use_gb_att else None
    gb_ff = nc.declare_dram_parameter("gb_ff", [2, H], f32, isOutput=False) if use_gb_ff else None

    out = nc.declare_dram_parameter("out", [TOK, NL], f32, isOutput=True)

    from contextlib import ExitStack
    with tile.TileContext(nc) as tc, ExitStack() as ctx:
        const = ctx.enter_context(tc.tile_pool(name="const", bufs=1))
        wpool = ctx.enter_context(tc.tile_pool(name="weights", bufs=1))
        span_sb = ctx.enter_context(tc.tile_pool(name="span_sb", bufs=2))
        masks = ctx.enter_context(tc.tile_pool(name="masks", bufs=2))
        chunks = ctx.enter_context(tc.tile_pool(name="chunks", bufs=3))
        mega = ctx.enter_context(tc.tile_pool(name="mega", bufs=2))
        xtp = ctx.enter_context(tc.tile_pool(name="xtp", bufs=2))
        h1pool = ctx.enter_context(tc.tile_pool(name="h1pool", bufs=1))
        ctpool = ctx.enter_context(tc.tile_pool(name="ctpool", bufs=2))
        stats = ctx.enter_context(tc.tile_pool(name="stats", bufs=4))
        outsb = ctx.enter_context(tc.tile_pool(name="outsb", bufs=3))

        # PSUM budget (8 banks): aux 4 + ps2 2 + ps1 2
        pp_aux = ctx.enter_context(tc.tile_pool(name="pp_aux", bufs=4, space="PSUM"))
        pp_mm1 = ctx.enter_context(tc.tile_pool(name="pp_mm1", bufs=2, space="PSUM"))
        pp_mm2 = ctx.enter_context(tc.tile_pool(name="pp_mm2", bufs=2, space="PSUM"))

        # ---- persistent constants / weights ----
        ident = const.tile([P, P], f16)
        make_identity(nc, ident)
        eps_t = const.tile([P, 1], f32)
        nc.vector.memset(eps_t, EPS)
        iota_l_sb = const.tile([P, L], f32)
        nc.gpsimd.dma_start(out=iota_l_sb, in_=iota_l[None, :].to_broadcast([P, L]))
        iota_t_sb = const.tile([P, P], f32)
        nc.gpsimd.dma_start(out=iota_t_sb, in_=iota_t[None, :].to_broadcast([P, P]))

        temb_sb = wpool.tile([P, H], f16)
        if T < P:
            nc.vector.memset(temb_sb, 0.0)
        nc.sync.dma_start(out=temb_sb[:T, :], in_=temb[:, :])
        wout_sb = wpool.tile([P, KSUB, NL], f16)
        nc.sync.dma_start(out=wout_sb, in_=wout.rearrange("(s p) n -> p s n", p=P))
        w1_sb = wpool.tile([P, KSUB, I], f16)
        nc.sync.dma_start(out=w1_sb, in_=w1.rearrange("(s p) i -> p s i", p=P))
        w2_sb = wpool.tile([P, ISUB, H], f16)
        nc.scalar.dma_start(out=w2_sb, in_=w2.rearrange("(s p) h -> p s h", p=P))

        b1_sb = None
        if b1 is not None:
            b1_sb = wpool.tile([P, ISUB], f32)
            nc.sync.dma_start(out=b1_sb, in_=b1.rearrange("(s p) -> p s", p=P))
        b2_sb = None
        if b2 is not None:
            b2_sb = wpool.tile([P, H], f32)
            nc.gpsimd.dma_start(out=b2_sb, in_=b2[None, :].to_broadcast([P, H]))
        bout_sb = None
        if bout is not None:
            bout_sb = wpool.tile([P, NL], f32)
            nc.gpsimd.dma_start(out=bout_sb, in_=bout[None, :].to_broadcast([P, NL]))
        gb_att_sb = None
        if gb_att is not None:
            gb_att_sb = wpool.tile([P, 2, H], f32)
            nc.gpsimd.dma_start(out=gb_att_sb, in_=gb_att[None, :, :].to_broadcast([P, 2, H]))
        gb_ff_sb = None
        if gb_ff is not None:
            gb_ff_sb = wpool.tile([P, 2, H], f32)
            nc.gpsimd.dma_start(out=gb_ff_sb, in_=gb_ff[None, :, :].to_broadcast([P, 2, H]))

        # rotating per-batch state (allocated by the prep stage)
        state = {}

        def emit_count(bp):
            """Span masks + count matmul -> countT_sb [128, L] f16 for batch bp."""
            cnt_psum = pp_aux.tile([P, L], f32, tag="aux")
            for st in range(nt_span):
                sp_t = span_sb.tile([P, 3], f32, tag="spans")
                nc.sync.dma_start(out=sp_t, in_=spans[bp, st, :, :])
                s_t, e_t, g_t = sp_t[:, 0:1], sp_t[:, 1:2], sp_t[:, 2:3]
                ge = masks.tile([P, L], f32, tag="ge")
                nc.vector.tensor_tensor(
                    out=ge, in0=iota_l_sb, in1=s_t.to_broadcast([P, L]),
                    op=mybir.AluOpType.is_ge)
                lt = masks.tile([P, L], f32, tag="lt")
                nc.vector.tensor_tensor(
                    out=lt, in0=iota_l_sb, in1=e_t.to_broadcast([P, L]),
                    op=mybir.AluOpType.is_lt)
                in_span = masks.tile([P, L], f16, tag="in_span")
                nc.vector.tensor_tensor(
                    out=in_span, in0=ge, in1=lt, op=mybir.AluOpType.mult)
                onehot = masks.tile([P, P], f16, tag="onehot")
                nc.vector.tensor_tensor(
                    out=onehot, in0=iota_t_sb, in1=g_t.to_broadcast([P, P]),
                    op=mybir.AluOpType.is_equal)
                nc.tensor.matmul(cnt_psum, lhsT=onehot, rhs=in_span,
                                 start=(st == 0), stop=(st == nt_span - 1))
            countT = ctpool.tile([P, L], f16, tag="countT")
            nc.vector.tensor_copy(out=countT, in_=cnt_psum)
            state[bp] = {"countT": countT}

        def prep_head(bp, ci):
            """taged + LN1 chain (PE: 2 small matmuls; rest DVE)."""
            st_b = state[bp]
            if ci == 0:
                st_b["xn_f32"] = mega.tile([P, 4, H], f32, tag="xn_f32", name="xn_f32")
                st_b["xT"] = xtp.tile([P, KSUB, L], f16, tag="xT", name="xT")
            row0 = bp * L + ci * P
            tg_a = pp_aux.tile([P, NH], f32, tag="aux", name="tg_a")
            tg_b = pp_aux.tile([P, NH], f32, tag="aux", name="tg_b")
            csl = st_b["countT"][:, ci * P:(ci + 1) * P]
            nc.tensor.matmul(tg_a, lhsT=csl, rhs=temb_sb[:, :NH],
                             start=True, stop=True)
            nc.tensor.matmul(tg_b, lhsT=csl, rhs=temb_sb[:, NH:],
                             start=True, stop=True)
            we_t = chunks.tile([P, H], f32, tag="we")
            nc.sync.dma_start(out=we_t, in_=we[row0:row0 + P, :])
            xpre = chunks.tile([P, H], f32, tag="xpre")
            nc.vector.tensor_add(out=xpre[:, :NH], in0=we_t[:, :NH], in1=tg_a)
            nc.vector.tensor_add(out=xpre[:, NH:], in0=we_t[:, NH:], in1=tg_b)

            mean, rstd = _ln_stats(nc, stats, xpre, eps_t)
            xn32 = st_b["xn_f32"][:, ci, :]
            nc.vector.tensor_scalar(
                out=xn32, in0=xpre, scalar1=mean, scalar2=rstd,
                op0=mybir.AluOpType.subtract, op1=mybir.AluOpType.mult)
            if gb_att_sb is not None:
                nc.vector.tensor_mul(out=xn32, in0=xn32, in1=gb_att_sb[:, 0, :])
                nc.vector.tensor_add(out=xn32, in0=xn32, in1=gb_att_sb[:, 1, :])
            xn16 = chunks.tile([P, H], f16, tag="xn16")
            nc.vector.tensor_copy(out=xn16, in_=xn32)
            st_b[("xn16", ci)] = xn16

        def prep_tail(bp, ci):
            """PE transposes of xn16 into xT (emitted after a matmul burst
            so the LN1 chain has already finished on the DVE)."""
            st_b = state[bp]
            xn16 = st_b.pop(("xn16", ci))
            tp_x = pp_aux.tile([P, H], f16, tag="aux", name="tp_x")
            for k in range(KSUB):
                nc.tensor.transpose(tp_x[:, k * P:(k + 1) * P],
                                    xn16[:, k * P:(k + 1) * P], ident)
            nc.vector.tensor_copy(
                out=st_b["xT"][:, :, ci * P:(ci + 1) * P],
                in_=tp_x.rearrange("p (k t) -> p k t", t=P))

        def emit_mm1(b):
            st_b = state[b]
            h1T = h1pool.tile([P, ISUB, L], f16, tag="h1T")
            st_b["h1T"] = h1T
            xT = st_b["xT"]
            for isub in range(ISUB):
                ps1 = pp_mm1.tile([P, L], f32, tag="ps1")
                for k in range(KSUB):
                    nc.tensor.matmul(
                        ps1, lhsT=w1_sb[:, k, isub * P:(isub + 1) * P],
                        rhs=xT[:, k, :], start=(k == 0), stop=(k == KSUB - 1))
                if b1_sb is not None:
                    nc.vector.tensor_scalar(
                        out=h1T[:, isub, :], in0=ps1,
                        scalar1=b1_sb[:, isub:isub + 1], scalar2=0.0,
                        op0=mybir.AluOpType.add, op1=mybir.AluOpType.max)
                else:
                    nc.vector.tensor_scalar(
                        out=h1T[:, isub, :], in0=ps1, scalar1=0.0, scalar2=None,
                        op0=mybir.AluOpType.max)
                if isub == 1 and b + 1 < BPC:
                    emit_count(b + 1)

        def emit_mm2(b, ci):
            st_b = state[b]
            h1T = st_b["h1T"]
            ps2a = pp_mm2.tile([P, NH], f32, tag="ps2", name="ps2a")
            for isub in range(ISUB):
                nc.tensor.matmul(ps2a, lhsT=h1T[:, isub, ci * P:(ci + 1) * P],
                                 rhs=w2_sb[:, isub, :NH],
                                 start=(isub == 0), stop=(isub == ISUB - 1))
            ps2b = pp_mm2.tile([P, NH], f32, tag="ps2", name="ps2b")
            for isub in range(ISUB):
                nc.tensor.matmul(ps2b, lhsT=h1T[:, isub, ci * P:(ci + 1) * P],
                                 rhs=w2_sb[:, isub, NH:],
                                 start=(isub == 0), stop=(isub == ISUB - 1))
            st_b[("ps2", ci)] = (ps2a, ps2b)

        def out_dve(b, ci):
            """residual + LN2 on the DVE (no PE work)."""
            st_b = state[b]
            ps2a, ps2b = st_b.pop(("ps2", ci))
            xn32 = st_b["xn_f32"][:, ci, :]
            h2 = chunks.tile([P, H], f32, tag="h2")
            nc.vector.tensor_add(out=h2[:, :NH], in0=ps2a, in1=xn32[:, :NH])
            nc.vector.tensor_add(out=h2[:, NH:], in0=ps2b, in1=xn32[:, NH:])
            if b2_sb is not None:
                nc.vector.tensor_add(out=h2, in0=h2, in1=b2_sb)

            mean2, rstd2 = _ln_stats(nc, stats, h2, eps_t)
            h2n = chunks.tile([P, H], f16, tag="h2n")
            nc.vector.tensor_scalar(
                out=h2n, in0=h2, scalar1=mean2, scalar2=rstd2,
                op0=mybir.AluOpType.subtract, op1=mybir.AluOpType.mult)
            if gb_ff_sb is not None:
                nc.vector.tensor_mul(out=h2n, in0=h2n, in1=gb_ff_sb[:, 0, :])
                nc.vector.tensor_add(out=h2n, in0=h2n, in1=gb_ff_sb[:, 1, :])
            st_b[("h2n", ci)] = h2n

        def out_pe(b, ci):
            """h2n transposes + output projection (emitted after a matmul
            burst so LN2 has already finished on the DVE)."""
            st_b = state[b]
            h2n = st_b.pop(("h2n", ci))
            row0 = b * L + ci * P
            tp_h = pp_aux.tile([P, H], f16, tag="aux", name="tp_h")
            for k in range(KSUB):
                nc.tensor.transpose(tp_h[:, k * P:(k + 1) * P],
                                    h2n[:, k * P:(k + 1) * P], ident)
            h2nT = chunks.tile([P, H], f16, tag="h2nT")
            nc.vector.tensor_copy(out=h2nT, in_=tp_h)
            ps3 = pp_aux.tile([P, NL], f32, tag="aux", name="ps3")
            for k in range(KSUB):
                nc.tensor.matmul(ps3, lhsT=h2nT[:, k * P:(k + 1) * P],
                                 rhs=wout_sb[:, k, :],
                                 start=(k == 0), stop=(k == KSUB - 1))
            o_t = outsb.tile([P, NL], f32, tag="o")
            if bout_sb is not None:
                nc.vector.tensor_add(out=o_t, in0=ps3, in1=bout_sb)
            else:
                nc.vector.tensor_copy(out=o_t, in_=ps3)
            nc.sync.dma_start(out=out[row0:row0 + P, :], in_=o_t)

        # ---- pipelined emission ----
        emit_count(0)
        for ci in range(4):
            prep_head(0, ci)
        for ci in range(4):
            prep_tail(0, ci)
        pending = []
        for b in range(BPC):
            emit_mm1(b)
            for ci in range(4):
                if pending:
                    out_dve(*pending[0])
                if b + 1 < BPC:
                    prep_head(b + 1, ci)
                emit_mm2(b, ci)
                if b + 1 < BPC:
                    prep_tail(b + 1, ci)
                if pending:
                    out_pe(*pending.pop(0))
                pending.append((b, ci))
            if b > 1:
                del state[b - 2]
        while pending:
            b, ci = pending.pop(0)
            out_dve(b, ci)
            out_pe(b, ci)

    _split_multi_waits(nc)
    return nc


def _ln_stats(nc, stats_pool, x, eps_t):
    """mean/rstd over the free dim (H=768) via bn_stats in 256-wide groups."""
    sub = 256
    n_sub = H // sub
    st = stats_pool.tile([P, n_sub, 6], f32, tag="bn_st")
    xg = x.rearrange("p (n s) -> p n s", s=sub)
    for i in range(n_sub):
        nc.vector.bn_stats(out=st[:, i, :], in_=xg[:, i, :])
    mv = stats_pool.tile([P, 2], f32, tag="bn_mv")
    nc.vector.bn_aggr(out=mv, in_=st)
    rstd = stats_pool.tile([P, 1], f32, tag="rstd")
    nc.scalar.activation(out=rstd, in_=mv[:, 1:2],
                         func=mybir.ActivationFunctionType.Sqrt,
                         bias=eps_t, scale=1.0)
    nc.vector.reciprocal(out=rstd, in_=rstd)
    return mv[:, 0:1], rstd


def _split_multi_waits(nc, max_waits=1):
    """walrus codegen in this toolchain accepts at most one sync wait per
    compute instruction; hoist extras onto same-engine NoOps just before."""
    n_nops = 0
    for f in nc.m.functions:
        for blk in f.blocks:
            insts = blk.instructions
            out = []
            changed = False
            for inst in insts:
                si = getattr(inst, "sync_info", None)
                waits = list(si.on_wait) if si is not None and si.on_wait else []
                if len(waits) > max_waits:
                    for w in waits[:-max_waits]:
                        nop = mybir.InstNoOp(
                            name=f"W-split-{n_nops}", ins=[], outs=[])
                        nop.engine = inst.engine
                        nop.sync_info = mybir.SyncInfo(on_wait=[w], on_update=[])
                        out.append(nop)
                        n_nops += 1
                    inst.sync_info = mybir.SyncInfo(
                        on_wait=waits[-max_waits:], on_update=list(si.on_update))
                    changed = True
                out.append(inst)
            if changed:
                blk.instructions = out
    return n_nops


_BUILT = {}


def _prep_inputs(word_embedding, tag_emb, w1, b1, w2, b2, g_att, be_att,
                 g_ff, be_ff, w_out, b_out, span_b, span_tag, span_start,
                 span_end):
    """Host-side sharding: bucket spans by batch, cast weights, build in_maps."""
    we = np.ascontiguousarray(np.asarray(word_embedding, np.float32))
    sb = np.asarray(span_b).astype(np.int64)
    stg = np.asarray(span_tag).astype(np.int64)
    ss = np.asarray(span_start).astype(np.int64)
    se = np.asarray(span_end).astype(np.int64)

    counts = np.bincount(sb, minlength=B)
    nt_span = max(1, math.ceil(counts.max() / P))
    smax = nt_span * P
    spans = np.zeros((B, smax, 3), np.float32)
    spans[:, :, 2] = -1.0  # tag -1 never matches iota_t
    for b in range(B):
        idx = np.flatnonzero(sb == b)
        n = len(idx)
        spans[b, :n, 0] = ss[idx]
        spans[b, :n, 1] = se[idx]
        spans[b, :n, 2] = stg[idx]

    w1h = np.asarray(w1, np.float32).astype(np.float16)
    w2h = np.asarray(w2, np.float32).astype(np.float16)
    wouth = np.asarray(w_out, np.float32).astype(np.float16)
    tembh = (np.asarray(tag_emb, np.float32) * RATE).astype(np.float16)

    b1_ = np.asarray(b1, np.float32)
    b2_ = np.asarray(b2, np.float32)
    bout_ = np.asarray(b_out, np.float32)
    ga = np.asarray(g_att, np.float32)
    ba = np.asarray(be_att, np.float32)
    gf = np.asarray(g_ff, np.float32)
    bf = np.asarray(be_ff, np.float32)
    use_b1 = bool(np.any(b1_ != 0))
    use_b2 = bool(np.any(b2_ != 0))
    use_bout = bool(np.any(bout_ != 0))
    use_gb_att = bool(np.any(ga != 1) or np.any(ba != 0))
    use_gb_ff = bool(np.any(gf != 1) or np.any(bf != 0))

    iota_l = np.arange(L, dtype=np.float32)
    iota_t = np.arange(P, dtype=np.float32)

    in_maps = []
    for c in range(N_CORES):
        b0 = c * BPC
        m = dict(
            we=we[b0:b0 + BPC].reshape(TOK, H),
            w1=w1h, w2=w2h, wout=wouth, temb=tembh,
            spans=spans[b0:b0 + BPC].reshape(BPC, nt_span, P, 3),
            iota_l=iota_l, iota_t=iota_t,
        )
        if use_b1:
            m["b1"] = b1_
        if use_b2:
            m["b2"] = b2_
        if use_bout:
            m["bout"] = bout_
        if use_gb_att:
            m["gb_att"] = np.stack([ga, ba])
        if use_gb_ff:
            m["gb_ff"] = np.stack([gf, bf])
        in_maps.append(m)

    key = (nt_span, use_b1, use_b2, use_bout, use_gb_att, use_gb_ff)
    return key, in_maps


def kernel(**inputs):
    key, in_maps = _prep_inputs(**inputs)
    if key not in _BUILT:
        _BUILT[key] = build_kernel(*key)
    nc = _BUILT[key]
    res = run_bass_kernel_spmd(nc, in_maps, core_ids=list(range(N_CORES)))
    outs = [res.results[c]["out"].reshape(BPC, L, NL) for c in range(N_CORES)]
    return np.concatenate(outs, axis=0).astype(np.float32)


# revision 13
# speedup vs baseline: 1.6189x; 1.0146x over previous
"""Trainium2 Bass kernel for nn_Estor_raw_45595372814583.

Reference computation (B=64, L=512, H=768, I=3072, T=50, NL=9, S=4096):
    taged[b, s:e, :] += tag_emb[tag]      for each span (b, tag, s, e)
    x   = LN(word_embedding + 0.5 * taged) * g_att + be_att
    h   = relu(x @ w1 + b1) @ w2 + b2 + x
    h   = LN(h) * g_ff + be_ff
    out = h @ w_out + b_out               # [B, L, 9]

Strategy: data-parallel over batch across 8 cores (8 batches each). The
span scatter is computed on-device as two small matmuls per batch:
    in_span[s, l] = (l >= start_s) & (l < end_s)        (DVE compares vs iota)
    onehot[s, t]  = (tag_s == t)
    countT[t, l]  = onehot.T @ in_span                  (PE)
    taged[l, :]   = countT[:, l].T @ (0.5 * tag_emb)    (PE)
The FFN runs in fp16 on the PE with fp32 PSUM accumulation; LayerNorm
stats use bn_stats/bn_aggr on the DVE in fp32. Activation transposes go
through the (otherwise idle) DMA XBAR. Batches are software-pipelined:
while batch b runs its second FFN matmul, batch b+1's scatter+LN1 chain
executes on the DVE, and each chunk's output stage is delayed by one
chunk so its LN2 latency hides under the next chunk's matmuls.
"""

import math
import os
import sys

import numpy as np

for _p in ("/opt/trn_rl_repo", "/opt/trn_rl_repo/concourse"):
    if _p not in sys.path and os.path.isdir(_p):
        sys.path.insert(0, _p)

import concourse.bass as bass
import concourse.mybir as mybir
import concourse.tile as tile
from concourse.bass_utils import run_bass_kernel_spmd
from concourse.masks import make_identity

B, L, H, I, T, NL = 64, 512, 768, 3072, 50, 9
RATE = 0.5
EPS = 1e-12
P = 128
N_CORES = 8
BPC = B // N_CORES          # batches per core
TOK = BPC * L               # tokens per core
KSUB = H // P               # 6   k-subtiles over H
ISUB = I // P               # 24  subtiles over I
NH = H // 2                 # 384 n-half for H-wide psum outputs

f32 = mybir.dt.float32
f16 = mybir.dt.float16


def build_kernel(nt_span: int, use_b1: bool, use_b2: bool, use_bout: bool,
                 use_gb_att: bool, use_gb_ff: bool):
    """Build the SPMD Bass program (same program on all 8 cores).

    nt_span: number of 128-span tiles per batch (spans padded to nt_span*128).
    """
    nc = bass.Bass()

    we = nc.declare_dram_parameter("we", [TOK, H], f32, isOutput=False)
    w1 = nc.declare_dram_parameter("w1", [H, I], f16, isOutput=False)
    w2 = nc.declare_dram_parameter("w2", [I, H], f16, isOutput=False)
    wout = nc.declare_dram_parameter("wout", [H, NL], f16, isOutput=False)
    temb = nc.declare_dram_parameter("temb", [T, H], f16, isOutput=False)
    spans = nc.declare_dram_parameter("spans", [BPC, nt_span, P, 3], f32, isOutput=False)
    iota_l = nc.declare_dram_parameter("iota_l", [L], f32, isOutput=False)
    iota_t = nc.declare_dram_parameter("iota_t", [P], f32, isOutput=False)
    b1 = nc.declare_dram_parameter("b1", [I], f32, isOutput=False) if use_b1 else None
    b2 = nc.declare_dram_parameter("b2", [H], f32, isOutput=False) if use_b2 else None
    bout = nc.declare_dram_parameter("bout", [NL], f32, isOutput=False) if use_bout else None
    gb_att = nc.declare_dram_parameter("gb_att", [2, H], f32, isOutput=False) if use_gb_att else None
    gb_ff = nc.declare_dram_parameter("gb_ff", [2, H], f32, isOutput=False) if use_gb_ff else None

    out = nc.declare_dram_parameter("out", [TOK, NL], f32, isOutput=True)

    from contextlib import ExitStack
    with tile.TileContext(nc) as tc, ExitStack() as ctx:
        const = ctx.enter_context(tc.tile_pool(name="const", bufs=1))
        wpool = ctx.enter_context(tc.tile_pool(name="weights", bufs=1))
        span_sb = ctx.enter_context(tc.tile_pool(name="span_sb", bufs=2))
        masks = ctx.enter_context(tc.tile_pool(name="masks", bufs=2))
        chunks = ctx.enter_context(tc.tile_pool(name="chunks", bufs=3))
        mega = ctx.enter_context(tc.tile_pool(name="mega", bufs=2))
        xtp = ctx.enter_context(tc.tile_pool(name="xtp", bufs=2))
        h1pool = ctx.enter_context(tc.tile_pool(name="h1pool", bufs=1))
        ctpool = ctx.enter_context(tc.tile_pool(name="ctpool", bufs=2))
        stats = ctx.enter_context(tc.tile_pool(name="stats", bufs=4))
        outsb = ctx.enter_context(tc.tile_pool(name="outsb", bufs=3))

        # PSUM budget (8 banks): aux 4 + ps2 2 + ps1 2
        pp_aux = ctx.enter_context(tc.tile_pool(name="pp_aux", bufs=4, space="PSUM"))
        pp_mm1 = ctx.enter_context(tc.tile_pool(name="pp_mm1", bufs=2, space="PSUM"))
        pp_mm2 = ctx.enter_context(tc.tile_pool(name="pp_mm2", bufs=2, space="PSUM"))

        # ---- persistent constants / weights ----
        ident = const.tile([P, P], f16)
        make_identity(nc, ident)
        eps_t = const.tile([P, 1], f32)
        nc.vector.memset(eps_t, EPS)
        iota_l_sb = const.tile([P, L], f32)
        nc.gpsimd.dma_start(out=iota_l_sb, in_=iota_l[None, :].to_broadcast([P, L]))
        iota_t_sb = const.tile([P, P], f32)
        nc.gpsimd.dma_start(out=iota_t_sb, in_=iota_t[None, :].to_broadcast([P, P]))

        temb_sb = wpool.tile([P, H], f16)
        if T < P:
            nc.vector.memset(temb_sb, 0.0)
        nc.sync.dma_start(out=temb_sb[:T, :], in_=temb[:, :])
        wout_sb = wpool.tile([P, KSUB, NL], f16)
        nc.sync.dma_start(out=wout_sb, in_=wout.rearrange("(s p) n -> p s n", p=P))
        w1_sb = wpool.tile([P, KSUB, I], f16)
        nc.sync.dma_start(out=w1_sb, in_=w1.rearrange("(s p) i -> p s i", p=P))
        w2_sb = wpool.tile([P, ISUB, H], f16)
        nc.scalar.dma_start(out=w2_sb, in_=w2.rearrange("(s p) h -> p s h", p=P))

        b1_sb = None
        if b1 is not None:
            b1_sb = wpool.tile([P, ISUB], f32)
            nc.sync.dma_start(out=b1_sb, in_=b1.rearrange("(s p) -> p s", p=P))
        b2_sb = None
        if b2 is not None:
            b2_sb = wpool.tile([P, H], f32)
            nc.gpsimd.dma_start(out=b2_sb, in_=b2[None, :].to_broadcast([P, H]))
        bout_sb = None
        if bout is not None:
            bout_sb = wpool.tile([P, NL], f32)
            nc.gpsimd.dma_start(out=bout_sb, in_=bout[None, :].to_broadcast([P, NL]))
        gb_att_sb = None
        if gb_att is not None:
            gb_att_sb = wpool.tile([P, 2, H], f32)
            nc.gpsimd.dma_start(out=gb_att_sb, in_=gb_att[None, :, :].to_broadcast([P, 2, H]))
        gb_ff_sb = None
        if gb_ff is not None:
            gb_ff_sb = wpool.tile([P, 2, H], f32)
            nc.gpsimd.dma_start(out=gb_ff_sb, in_=gb_ff[None, :, :].to_broadcast([P, 2, H]))

        # rotating per-batch state (allocated by the prep stage)
        state = {}

        def emit_count(bp):
            """Span masks + count matmul -> countT_sb [128, L] f16 for batch bp."""
            cnt_psum = pp_aux.tile([P, L], f32, tag="aux")
            for st in range(nt_span):
                sp_t = span_sb.tile([P, 3], f32, tag="spans")
                nc.gpsimd.dma_start(out=sp_t, in_=spans[bp, st, :, :])
                s_t, e_t, g_t = sp_t[:, 0:1], sp_t[:, 1:2], sp_t[:, 2:3]
                ge = masks.tile([P, L], f32, tag="ge")
                nc.vector.tensor_tensor(
                    out=ge, in0=iota_l_sb, in1=s_t.to_broadcast([P, L]),
                    op=mybir.AluOpType.is_ge)
                lt = masks.tile([P, L], f32, tag="lt")
                nc.vector.tensor_tensor(
                    out=lt, in0=iota_l_sb, in1=e_t.to_broadcast([P, L]),
                    op=mybir.AluOpType.is_lt)
                in_span = masks.tile([P, L], f16, tag="in_span")
                nc.vector.tensor_tensor(
                    out=in_span, in0=ge, in1=lt, op=mybir.AluOpType.mult)
                onehot = masks.tile([P, P], f16, tag="onehot")
                nc.vector.tensor_tensor(
                    out=onehot, in0=iota_t_sb, in1=g_t.to_broadcast([P, P]),
                    op=mybir.AluOpType.is_equal)
                nc.tensor.matmul(cnt_psum, lhsT=onehot, rhs=in_span,
                                 start=(st == 0), stop=(st == nt_span - 1))
            countT = ctpool.tile([P, L], f16, tag="countT")
            nc.vector.tensor_copy(out=countT, in_=cnt_psum)
            state[bp] = {"countT": countT}

        def prep_head(bp, ci):
            """taged + LN1 chain (PE: 2 small matmuls; rest DVE)."""
            st_b = state[bp]
            if ci == 0:
                st_b["xn_f32"] = mega.tile([P, 4, H], f32, tag="xn_f32", name="xn_f32")
                st_b["xT"] = xtp.tile([P, KSUB, L], f16, tag="xT", name="xT")
            row0 = bp * L + ci * P
            tg_a = pp_aux.tile([P, NH], f32, tag="aux", name="tg_a")
            tg_b = pp_aux.tile([P, NH], f32, tag="aux", name="tg_b")
            csl = st_b["countT"][:, ci * P:(ci + 1) * P]
            nc.tensor.matmul(tg_a, lhsT=csl, rhs=temb_sb[:, :NH],
                             start=True, stop=True)
            nc.tensor.matmul(tg_b, lhsT=csl, rhs=temb_sb[:, NH:],
                             start=True, stop=True)
            we_t = chunks.tile([P, H], f32, tag="we")
            nc.gpsimd.dma_start(out=we_t, in_=we[row0:row0 + P, :])
            xpre = chunks.tile([P, H], f32, tag="xpre")
            nc.vector.tensor_add(out=xpre[:, :NH], in0=we_t[:, :NH], in1=tg_a)
            nc.vector.tensor_add(out=xpre[:, NH:], in0=we_t[:, NH:], in1=tg_b)

            mean, rstd = _ln_stats(nc, stats, xpre, eps_t)
            xn32 = st_b["xn_f32"][:, ci, :]
            nc.vector.tensor_scalar(
                out=xn32, in0=xpre, scalar1=mean, scalar2=rstd,
                op0=mybir.AluOpType.subtract, op1=mybir.AluOpType.mult)
            if gb_att_sb is not None:
                nc.vector.tensor_mul(out=xn32, in0=xn32, in1=gb_att_sb[:, 0, :])
                nc.vector.tensor_add(out=xn32, in0=xn32, in1=gb_att_sb[:, 1, :])
            xn16 = chunks.tile([P, H], f16, tag="xn16")
            nc.vector.tensor_copy(out=xn16, in_=xn32)
            st_b[("xn16", ci)] = xn16

        def prep_tail(bp, ci):
            """PE transposes of xn16 into xT (emitted after a matmul burst
            so the LN1 chain has already finished on the DVE)."""
            st_b = state[bp]
            xn16 = st_b.pop(("xn16", ci))
            tp_x = pp_aux.tile([P, H], f16, tag="aux", name="tp_x")
            for k in range(KSUB):
                nc.tensor.transpose(tp_x[:, k * P:(k + 1) * P],
                                    xn16[:, k * P:(k + 1) * P], ident)
            nc.vector.tensor_copy(
                out=st_b["xT"][:, :, ci * P:(ci + 1) * P],
                in_=tp_x.rearrange("p (k t) -> p k t", t=P))

        def emit_mm1(b):
            st_b = state[b]
            h1T = h1pool.tile([P, ISUB, L], f16, tag="h1T")
            st_b["h1T"] = h1T
            xT = st_b["xT"]
            for isub in range(ISUB):
                ps1 = pp_mm1.tile([P, L], f32, tag="ps1")
                for k in range(KSUB):
                    nc.tensor.matmul(
                        ps1, lhsT=w1_sb[:, k, isub * P:(isub + 1) * P],
                        rhs=xT[:, k, :], start=(k == 0), stop=(k == KSUB - 1))
                if b1_sb is not None:
                    nc.vector.tensor_scalar(
                        out=h1T[:, isub, :], in0=ps1,
                        scalar1=b1_sb[:, isub:isub + 1], scalar2=0.0,
                        op0=mybir.AluOpType.add, op1=mybir.AluOpType.max)
                else:
                    nc.vector.tensor_scalar(
                        out=h1T[:, isub, :], in0=ps1, scalar1=0.0, scalar2=None,
                        op0=mybir.AluOpType.max)
                if isub == 1 and b + 1 < BPC:
                    emit_count(b + 1)

        def emit_mm2(b, ci):
            st_b = state[b]
            h1T = st_b["h1T"]
            ps2a = pp_mm2.tile([P, NH], f32, tag="ps2", name="ps2a")
            for isub in range(ISUB):
                nc.tensor.matmul(ps2a, lhsT=h1T[:, isub, ci * P:(ci + 1) * P],
                                 rhs=w2_sb[:, isub, :NH],
                                 start=(isub == 0), stop=(isub == ISUB - 1))
            ps2b = pp_mm2.tile([P, NH], f32, tag="ps2", name="ps2b")
            for isub in range(ISUB):
                nc.tensor.matmul(ps2b, lhsT=h1T[:, isub, ci * P:(ci + 1) * P],
                                 rhs=w2_sb[:, isub, NH:],
                                 start=(isub == 0), stop=(isub == ISUB - 1))
            st_b[("ps2", ci)] = (ps2a, ps2b)

        def out_dve(b, ci):
            """residual + LN2 on the DVE (no PE work)."""
            st_b = state[b]
            ps2a, ps2b = st_b.pop(("ps2", ci))
            xn32 = st_b["xn_f32"][:, ci, :]
            h2 = chunks.tile([P, H], f32, tag="h2")
            nc.vector.tensor_add(out=h2[:, :NH], in0=ps2a, in1=xn32[:, :NH])
            nc.vector.tensor_add(out=h2[:, NH:], in0=ps2b, in1=xn32[:, NH:])
            if b2_sb is not None:
                nc.vector.tensor_add(out=h2, in0=h2, in1=b2_sb)

            mean2, rstd2 = _ln_stats(nc, stats, h2, eps_t)
            h2n = chunks.tile([P, H], f16, tag="h2n")
            nc.vector.tensor_scalar(
                out=h2n, in0=h2, scalar1=mean2, scalar2=rstd2,
                op0=mybir.AluOpType.subtract, op1=mybir.AluOpType.mult)
            if gb_ff_sb is not None:
                nc.vector.tensor_mul(out=h2n, in0=h2n, in1=gb_ff_sb[:, 0, :])
                nc.vector.tensor_add(out=h2n, in0=h2n, in1=gb_ff_sb[:, 1, :])
            st_b[("h2n", ci)] = h2n

        def out_pe(b, ci):
            """h2n transposes + output projection (emitted after a matmul
            burst so LN2 has already finished on the DVE)."""
            st_b = state[b]
            h2n = st_b.pop(("h2n", ci))
            row0 = b * L + ci * P
            tp_h = pp_aux.tile([P, H], f16, tag="aux", name="tp_h")
            for k in range(KSUB):
                nc.tensor.transpose(tp_h[:, k * P:(k + 1) * P],
                                    h2n[:, k * P:(k + 1) * P], ident)
            h2nT = chunks.tile([P, H], f16, tag="h2nT")
            nc.vector.tensor_copy(out=h2nT, in_=tp_h)
            ps3 = pp_aux.tile([P, NL], f32, tag="aux", name="ps3")
            for k in range(KSUB):
                nc.tensor.matmul(ps3, lhsT=h2nT[:, k * P:(k + 1) * P],
                                 rhs=wout_sb[:, k, :],
                                 start=(k == 0), stop=(k == KSUB - 1))
            o_t = outsb.tile([P, NL], f32, tag="o")
            if bout_sb is not None:
                nc.vector.tensor_add(out=o_t, in0=ps3, in1=bout_sb)
            else:
                nc.vector.tensor_copy(out=o_t, in_=ps3)
            nc.sync.dma_start(out=out[row0:row0 + P, :], in_=o_t)

        # ---- pipelined emission ----
        emit_count(0)
        for ci in range(4):
            prep_head(0, ci)
        for ci in range(4):
            prep_tail(0, ci)
        pending = []
        for b in range(BPC):
            emit_mm1(b)
            for ci in range(4):
                if pending:
                    out_dve(*pending[0])
                if b + 1 < BPC:
                    prep_head(b + 1, ci)
                emit_mm2(b, ci)
                if b + 1 < BPC:
                    prep_tail(b + 1, ci)
                if pending:
                    out_pe(*pending.pop(0))
                pending.append((b, ci))
            if b > 1:
                del state[b - 2]
        while pending:
            b, ci = pending.pop(0)
            out_dve(b, ci)
            out_pe(b, ci)

    _split_multi_waits(nc)
    return nc


def _ln_stats(nc, stats_pool, x, eps_t):
    """mean/rstd over the free dim (H=768) via bn_stats in 256-wide groups."""
    sub = 256
    n_sub = H // sub
    st = stats_pool.tile([P, n_sub, 6], f32, tag="bn_st")
    xg = x.rearrange("p (n s) -> p n s", s=sub)
    for i in range(n_sub):
        nc.vector.bn_stats(out=st[:, i, :], in_=xg[:, i, :])
    mv = stats_pool.tile([P, 2], f32, tag="bn_mv")
    nc.vector.bn_aggr(out=mv, in_=st)
    rstd = stats_pool.tile([P, 1], f32, tag="rstd")
    nc.scalar.activation(out=rstd, in_=mv[:, 1:2],
                         func=mybir.ActivationFunctionType.Sqrt,
                         bias=eps_t, scale=1.0)
    nc.vector.reciprocal(out=rstd, in_=rstd)
    return mv[:, 0:1], rstd


def _split_multi_waits(nc, max_waits=1):
    """walrus codegen in this toolchain accepts at most one sync wait per
    compute instruction; hoist extras onto same-engine NoOps just before."""
    n_nops = 0
    for f in nc.m.functions:
        for blk in f.blocks:
            insts = blk.instructions
            out = []
            changed = False
            for inst in insts:
                si = getattr(inst, "sync_info", None)
                waits = list(si.on_wait) if si is not None and si.on_wait else []
                if len(waits) > max_waits:
                    for w in waits[:-max_waits]:
                        nop = mybir.InstNoOp(
                            name=f"W-split-{n_nops}", ins=[], outs=[])
                        nop.engine = inst.engine
                        nop.sync_info = mybir.SyncInfo(on_wait=[w], on_update=[])
                        out.append(nop)
                        n_nops += 1
                    inst.sync_info = mybir.SyncInfo(
                        on_wait=waits[-max_waits:], on_update=list(si.on_update))
                    changed = True
                out.append(inst)
            if changed:
                blk.instructions = out
    return n_nops


_BUILT = {}


def _prep_inputs(word_embedding, tag_emb, w1, b1, w2, b2, g_att, be_att,
                 g_ff, be_ff, w_out, b_out, span_b, span_tag, span_start,
                 span_end):
    """Host-side sharding: bucket spans by batch, cast weights, build in_maps."""
    we = np.ascontiguousarray(np.asarray(word_embedding, np.float32))
    sb = np.asarray(span_b).astype(np.int64)
    stg = np.asarray(span_tag).astype(np.int64)
    ss = np.asarray(span_start).astype(np.int64)
    se = np.asarray(span_end).astype(np.int64)

    counts = np.bincount(sb, minlength=B)
    nt_span = max(1, math.ceil(counts.max() / P))
    smax = nt_span * P
    spans = np.zeros((B, smax, 3), np.float32)
    spans[:, :, 2] = -1.0  # tag -1 never matches iota_t
    for b in range(B):
        idx = np.flatnonzero(sb == b)
        n = len(idx)
        spans[b, :n, 0] = ss[idx]
        spans[b, :n, 1] = se[idx]
        spans[b, :n, 2] = stg[idx]

    w1h = np.asarray(w1, np.float32).astype(np.float16)
    w2h = np.asarray(w2, np.float32).astype(np.float16)
    wouth = np.asarray(w_out, np.float32).astype(np.float16)
    tembh = (np.asarray(tag_emb, np.float32) * RATE).astype(np.float16)

    b1_ = np.asarray(b1, np.float32)
    b2_ = np.asarray(b2, np.float32)
    bout_ = np.asarray(b_out, np.float32)
    ga = np.asarray(g_att, np.float32)
    ba = np.asarray(be_att, np.float32)
    gf = np.asarray(g_ff, np.float32)
    bf = np.asarray(be_ff, np.float32)
    use_b1 = bool(np.any(b1_ != 0))
    use_b2 = bool(np.any(b2_ != 0))
    use_bout = bool(np.any(bout_ != 0))
    use_gb_att = bool(np.any(ga != 1) or np.any(ba != 0))
    use_gb_ff = bool(np.any(gf != 1) or np.any(bf != 0))

    iota_l = np.arange(L, dtype=np.float32)
    iota_t = np.arange(P, dtype=np.float32)

    in_maps = []
    for c in range(N_CORES):
        b0 = c * BPC
        m = dict(
            we=we[b0:b0 + BPC].reshape(TOK, H),
            w1=w1h, w2=w2h, wout=wouth, temb=tembh,
            spans=spans[b0:b0 + BPC].reshape(BPC, nt_span, P, 3),
            iota_l=iota_l, iota_t=iota_t,
        )
        if use_b1:
            m["b1"] = b1_
        if use_b2:
            m["b2"] = b2_
        if use_bout:
            m["bout"] = bout_
        if use_gb_att:
            m["gb_att"] = np.stack([ga, ba])
        if use_gb_ff:
            m["gb_ff"] = np.stack([gf, bf])
        in_maps.append(m)

    key = (nt_span, use_b1, use_b2, use_bout, use_gb_att, use_gb_ff)
    return key, in_maps


def kernel(**inputs):
    key, in_maps = _prep_inputs(**inputs)
    if key not in _BUILT:
        _BUILT[key] = build_kernel(*key)
    nc = _BUILT[key]
    res = run_bass_kernel_spmd(nc, in_maps, core_ids=list(range(N_CORES)))
    outs = [res.results[c]["out"].reshape(BPC, L, NL) for c in range(N_CORES)]
    return np.concatenate(outs, axis=0).astype(np.float32)


# revision 14
# speedup vs baseline: 1.6345x; 1.0097x over previous
"""Trainium2 Bass kernel for nn_Estor_raw_45595372814583.

Reference computation (B=64, L=512, H=768, I=3072, T=50, NL=9, S=4096):
    taged[b, s:e, :] += tag_emb[tag]      for each span (b, tag, s, e)
    x   = LN(word_embedding + 0.5 * taged) * g_att + be_att
    h   = relu(x @ w1 + b1) @ w2 + b2 + x
    h   = LN(h) * g_ff + be_ff
    out = h @ w_out + b_out               # [B, L, 9]

Strategy: data-parallel over batch across 8 cores (8 batches each). The
span scatter is computed on-device as two small matmuls per batch:
    in_span[s, l] = (l >= start_s) & (l < end_s)        (DVE compares vs iota)
    onehot[s, t]  = (tag_s == t)
    countT[t, l]  = onehot.T @ in_span                  (PE)
    taged[l, :]   = countT[:, l].T @ (0.5 * tag_emb)    (PE)
The FFN runs in fp16 on the PE with fp32 PSUM accumulation; LayerNorm
stats use bn_stats/bn_aggr on the DVE in fp32. Activation transposes go
through the (otherwise idle) DMA XBAR. Batches are software-pipelined:
while batch b runs its second FFN matmul, batch b+1's scatter+LN1 chain
executes on the DVE, and each chunk's output stage is delayed by one
chunk so its LN2 latency hides under the next chunk's matmuls.
"""

import math
import os
import sys

import numpy as np

for _p in ("/opt/trn_rl_repo", "/opt/trn_rl_repo/concourse"):
    if _p not in sys.path and os.path.isdir(_p):
        sys.path.insert(0, _p)

import concourse.bass as bass
import concourse.mybir as mybir
import concourse.tile as tile
from concourse.bass_utils import run_bass_kernel_spmd
from concourse.masks import make_identity

B, L, H, I, T, NL = 64, 512, 768, 3072, 50, 9
RATE = 0.5
EPS = 1e-12
P = 128
N_CORES = 8
BPC = B // N_CORES          # batches per core
TOK = BPC * L               # tokens per core
KSUB = H // P               # 6   k-subtiles over H
ISUB = I // P               # 24  subtiles over I
NH = H // 2                 # 384 n-half for H-wide psum outputs

f32 = mybir.dt.float32
f16 = mybir.dt.float16


def build_kernel(nt_span: int, use_b1: bool, use_b2: bool, use_bout: bool,
                 use_gb_att: bool, use_gb_ff: bool):
    """Build the SPMD Bass program (same program on all 8 cores).

    nt_span: number of 128-span tiles per batch (spans padded to nt_span*128).
    """
    nc = bass.Bass()

    we = nc.declare_dram_parameter("we", [TOK, H], f32, isOutput=False)
    w1 = nc.declare_dram_parameter("w1", [H, I], f16, isOutput=False)
    w2 = nc.declare_dram_parameter("w2", [I, H], f16, isOutput=False)
    wout = nc.declare_dram_parameter("wout", [H, NL], f16, isOutput=False)
    temb = nc.declare_dram_parameter("temb", [T, H], f16, isOutput=False)
    spans = nc.declare_dram_parameter("spans", [BPC, nt_span, P, 3], f32, isOutput=False)
    b1 = nc.declare_dram_parameter("b1", [I], f32, isOutput=False) if use_b1 else None
    b2 = nc.declare_dram_parameter("b2", [H], f32, isOutput=False) if use_b2 else None
    bout = nc.declare_dram_parameter("bout", [NL], f32, isOutput=False) if use_bout else None
    gb_att = nc.declare_dram_parameter("gb_att", [2, H], f32, isOutput=False) if use_gb_att else None
    gb_ff = nc.declare_dram_parameter("gb_ff", [2, H], f32, isOutput=False) if use_gb_ff else None

    out = nc.declare_dram_parameter("out", [TOK, NL], f32, isOutput=True)

    from contextlib import ExitStack
    with tile.TileContext(nc) as tc, ExitStack() as ctx:
        const = ctx.enter_context(tc.tile_pool(name="const", bufs=1))
        wpool = ctx.enter_context(tc.tile_pool(name="weights", bufs=1))
        span_sb = ctx.enter_context(tc.tile_pool(name="span_sb", bufs=2))
        masks = ctx.enter_context(tc.tile_pool(name="masks", bufs=2))
        chunks = ctx.enter_context(tc.tile_pool(name="chunks", bufs=3))
        mega = ctx.enter_context(tc.tile_pool(name="mega", bufs=2))
        xtp = ctx.enter_context(tc.tile_pool(name="xtp", bufs=2))
        h1pool = ctx.enter_context(tc.tile_pool(name="h1pool", bufs=1))
        ctpool = ctx.enter_context(tc.tile_pool(name="ctpool", bufs=2))
        stats = ctx.enter_context(tc.tile_pool(name="stats", bufs=4))
        outsb = ctx.enter_context(tc.tile_pool(name="outsb", bufs=3))

        # PSUM budget (8 banks): aux 4 + ps2 2 + ps1 2
        pp_aux = ctx.enter_context(tc.tile_pool(name="pp_aux", bufs=4, space="PSUM"))
        pp_mm1 = ctx.enter_context(tc.tile_pool(name="pp_mm1", bufs=2, space="PSUM"))
        pp_mm2 = ctx.enter_context(tc.tile_pool(name="pp_mm2", bufs=2, space="PSUM"))

        # ---- persistent constants / weights ----
        ident = const.tile([P, P], f16)
        make_identity(nc, ident)
        eps_t = const.tile([P, 1], f32)
        nc.vector.memset(eps_t, EPS)
        iota_i32 = const.tile([P, L], mybir.dt.int32)
        nc.gpsimd.iota(iota_i32, pattern=[[1, L]], channel_multiplier=0)
        iota_l_sb = const.tile([P, L], f32)
        nc.vector.tensor_copy(out=iota_l_sb, in_=iota_i32)
        iota_t_sb = const.tile([P, P], f32)
        nc.vector.tensor_copy(out=iota_t_sb, in_=iota_i32[:, :P])

        temb_sb = wpool.tile([P, H], f16)
        if T < P:
            nc.vector.memset(temb_sb, 0.0)
        nc.sync.dma_start(out=temb_sb[:T, :], in_=temb[:, :])
        wout_sb = wpool.tile([P, KSUB, NL], f16)
        nc.sync.dma_start(out=wout_sb, in_=wout.rearrange("(s p) n -> p s n", p=P))
        w1_sb = wpool.tile([P, KSUB, I], f16)
        nc.sync.dma_start(out=w1_sb, in_=w1.rearrange("(s p) i -> p s i", p=P))
        w2_sb = wpool.tile([P, ISUB, H], f16)
        nc.scalar.dma_start(out=w2_sb, in_=w2.rearrange("(s p) h -> p s h", p=P))

        b1_sb = None
        if b1 is not None:
            b1_sb = wpool.tile([P, ISUB], f32)
            nc.sync.dma_start(out=b1_sb, in_=b1.rearrange("(s p) -> p s", p=P))
        b2_sb = None
        if b2 is not None:
            b2_sb = wpool.tile([P, H], f32)
            nc.gpsimd.dma_start(out=b2_sb, in_=b2[None, :].to_broadcast([P, H]))
        bout_sb = None
        if bout is not None:
            bout_sb = wpool.tile([P, NL], f32)
            nc.gpsimd.dma_start(out=bout_sb, in_=bout[None, :].to_broadcast([P, NL]))
        gb_att_sb = None
        if gb_att is not None:
            gb_att_sb = wpool.tile([P, 2, H], f32)
            nc.gpsimd.dma_start(out=gb_att_sb, in_=gb_att[None, :, :].to_broadcast([P, 2, H]))
        gb_ff_sb = None
        if gb_ff is not None:
            gb_ff_sb = wpool.tile([P, 2, H], f32)
            nc.gpsimd.dma_start(out=gb_ff_sb, in_=gb_ff[None, :, :].to_broadcast([P, 2, H]))

        # rotating per-batch state (allocated by the prep stage)
        state = {}

        def emit_count(bp):
            """Span masks + count matmul -> countT_sb [128, L] f16 for batch bp."""
            cnt_psum = pp_aux.tile([P, L], f32, tag="aux")
            for st in range(nt_span):
                sp_t = span_sb.tile([P, 3], f32, tag="spans")
                nc.gpsimd.dma_start(out=sp_t, in_=spans[bp, st, :, :])
                s_t, e_t, g_t = sp_t[:, 0:1], sp_t[:, 1:2], sp_t[:, 2:3]
                ge = masks.tile([P, L], f32, tag="ge")
                nc.vector.tensor_tensor(
                    out=ge, in0=iota_l_sb, in1=s_t.to_broadcast([P, L]),
                    op=mybir.AluOpType.is_ge)
                lt = masks.tile([P, L], f32, tag="lt")
                nc.vector.tensor_tensor(
                    out=lt, in0=iota_l_sb, in1=e_t.to_broadcast([P, L]),
                    op=mybir.AluOpType.is_lt)
                in_span = masks.tile([P, L], f16, tag="in_span")
                nc.vector.tensor_tensor(
                    out=in_span, in0=ge, in1=lt, op=mybir.AluOpType.mult)
                onehot = masks.tile([P, P], f16, tag="onehot")
                nc.vector.tensor_tensor(
                    out=onehot, in0=iota_t_sb, in1=g_t.to_broadcast([P, P]),
                    op=mybir.AluOpType.is_equal)
                nc.tensor.matmul(cnt_psum, lhsT=onehot, rhs=in_span,
                                 start=(st == 0), stop=(st == nt_span - 1))
            countT = ctpool.tile([P, L], f16, tag="countT")
            nc.vector.tensor_copy(out=countT, in_=cnt_psum)
            state[bp] = {"countT": countT}

        def prep_head(bp, ci):
            """taged + LN1 chain (PE: 2 small matmuls; rest DVE)."""
            st_b = state[bp]
            if ci == 0:
                st_b["xn_f32"] = mega.tile([P, 4, H], f32, tag="xn_f32", name="xn_f32")
                st_b["xT"] = xtp.tile([P, KSUB, L], f16, tag="xT", name="xT")
            row0 = bp * L + ci * P
            tg_a = pp_aux.tile([P, NH], f32, tag="aux", name="tg_a")
            tg_b = pp_aux.tile([P, NH], f32, tag="aux", name="tg_b")
            csl = st_b["countT"][:, ci * P:(ci + 1) * P]
            nc.tensor.matmul(tg_a, lhsT=csl, rhs=temb_sb[:, :NH],
                             start=True, stop=True)
            nc.tensor.matmul(tg_b, lhsT=csl, rhs=temb_sb[:, NH:],
                             start=True, stop=True)
            we_t = chunks.tile([P, H], f32, tag="we")
            nc.gpsimd.dma_start(out=we_t, in_=we[row0:row0 + P, :])
            xpre = chunks.tile([P, H], f32, tag="xpre")
            nc.vector.tensor_add(out=xpre[:, :NH], in0=we_t[:, :NH], in1=tg_a)
            nc.vector.tensor_add(out=xpre[:, NH:], in0=we_t[:, NH:], in1=tg_b)

            mean, rstd = _ln_stats(nc, stats, xpre, eps_t)
            xn32 = st_b["xn_f32"][:, ci, :]
            nc.vector.tensor_scalar(
                out=xn32, in0=xpre, scalar1=mean, scalar2=rstd,
                op0=mybir.AluOpType.subtract, op1=mybir.AluOpType.mult)
            if gb_att_sb is not None:
                nc.vector.tensor_mul(out=xn32, in0=xn32, in1=gb_att_sb[:, 0, :])
                nc.vector.tensor_add(out=xn32, in0=xn32, in1=gb_att_sb[:, 1, :])
            xn16 = chunks.tile([P, H], f16, tag="xn16")
            nc.vector.tensor_copy(out=xn16, in_=xn32)
            st_b[("xn16", ci)] = xn16

        def prep_tail(bp, ci):
            """PE transposes of xn16 into xT (emitted after a matmul burst
            so the LN1 chain has already finished on the DVE)."""
            st_b = state[bp]
            xn16 = st_b.pop(("xn16", ci))
            tp_x = pp_aux.tile([P, H], f16, tag="aux", name="tp_x")
            for k in range(KSUB):
                nc.tensor.transpose(tp_x[:, k * P:(k + 1) * P],
                                    xn16[:, k * P:(k + 1) * P], ident)
            nc.vector.tensor_copy(
                out=st_b["xT"][:, :, ci * P:(ci + 1) * P],
                in_=tp_x.rearrange("p (k t) -> p k t", t=P))

        def emit_mm1(b):
            st_b = state[b]
            h1T = h1pool.tile([P, ISUB, L], f16, tag="h1T")
            st_b["h1T"] = h1T
            xT = st_b["xT"]
            for isub in range(ISUB):
                ps1 = pp_mm1.tile([P, L], f32, tag="ps1")
                for k in range(KSUB):
                    nc.tensor.matmul(
                        ps1, lhsT=w1_sb[:, k, isub * P:(isub + 1) * P],
                        rhs=xT[:, k, :], start=(k == 0), stop=(k == KSUB - 1))
                if b1_sb is not None:
                    nc.vector.tensor_scalar(
                        out=h1T[:, isub, :], in0=ps1,
                        scalar1=b1_sb[:, isub:isub + 1], scalar2=0.0,
                        op0=mybir.AluOpType.add, op1=mybir.AluOpType.max)
                else:
                    nc.vector.tensor_scalar(
                        out=h1T[:, isub, :], in0=ps1, scalar1=0.0, scalar2=None,
                        op0=mybir.AluOpType.max)
                if isub == 1 and b + 1 < BPC:
                    emit_count(b + 1)

        def emit_mm2(b, ci):
            st_b = state[b]
            h1T = st_b["h1T"]
            ps2a = pp_mm2.tile([P, NH], f32, tag="ps2", name="ps2a")
            for isub in range(ISUB):
                nc.tensor.matmul(ps2a, lhsT=h1T[:, isub, ci * P:(ci + 1) * P],
                                 rhs=w2_sb[:, isub, :NH],
                                 start=(isub == 0), stop=(isub == ISUB - 1))
            ps2b = pp_mm2.tile([P, NH], f32, tag="ps2", name="ps2b")
            for isub in range(ISUB):
                nc.tensor.matmul(ps2b, lhsT=h1T[:, isub, ci * P:(ci + 1) * P],
                                 rhs=w2_sb[:, isub, NH:],
                                 start=(isub == 0), stop=(isub == ISUB - 1))
            st_b[("ps2", ci)] = (ps2a, ps2b)

        def out_dve(b, ci):
            """residual + LN2 on the DVE (no PE work)."""
            st_b = state[b]
            ps2a, ps2b = st_b.pop(("ps2", ci))
            xn32 = st_b["xn_f32"][:, ci, :]
            h2 = chunks.tile([P, H], f32, tag="h2")
            nc.vector.tensor_add(out=h2[:, :NH], in0=ps2a, in1=xn32[:, :NH])
            nc.vector.tensor_add(out=h2[:, NH:], in0=ps2b, in1=xn32[:, NH:])
            if b2_sb is not None:
                nc.vector.tensor_add(out=h2, in0=h2, in1=b2_sb)

            mean2, rstd2 = _ln_stats(nc, stats, h2, eps_t)
            h2n = chunks.tile([P, H], f16, tag="h2n")
            nc.vector.tensor_scalar(
                out=h2n, in0=h2, scalar1=mean2, scalar2=rstd2,
                op0=mybir.AluOpType.subtract, op1=mybir.AluOpType.mult)
            if gb_ff_sb is not None:
                nc.vector.tensor_mul(out=h2n, in0=h2n, in1=gb_ff_sb[:, 0, :])
                nc.vector.tensor_add(out=h2n, in0=h2n, in1=gb_ff_sb[:, 1, :])
            st_b[("h2n", ci)] = h2n

        def out_pe(b, ci):
            """h2n transposes + output projection (emitted after a matmul
            burst so LN2 has already finished on the DVE)."""
            st_b = state[b]
            h2n = st_b.pop(("h2n", ci))
            row0 = b * L + ci * P
            tp_h = pp_aux.tile([P, H], f16, tag="aux", name="tp_h")
            for k in range(KSUB):
                nc.tensor.transpose(tp_h[:, k * P:(k + 1) * P],
                                    h2n[:, k * P:(k + 1) * P], ident)
            h2nT = chunks.tile([P, H], f16, tag="h2nT")
            nc.vector.tensor_copy(out=h2nT, in_=tp_h)
            ps3 = pp_aux.tile([P, NL], f32, tag="aux", name="ps3")
            for k in range(KSUB):
                nc.tensor.matmul(ps3, lhsT=h2nT[:, k * P:(k + 1) * P],
                                 rhs=wout_sb[:, k, :],
                                 start=(k == 0), stop=(k == KSUB - 1))
            o_t = outsb.tile([P, NL], f32, tag="o")
            if bout_sb is not None:
                nc.vector.tensor_add(out=o_t, in0=ps3, in1=bout_sb)
            else:
                nc.vector.tensor_copy(out=o_t, in_=ps3)
            nc.sync.dma_start(out=out[row0:row0 + P, :], in_=o_t)

        # ---- pipelined emission ----
        emit_count(0)
        for ci in range(4):
            prep_head(0, ci)
        for ci in range(4):
            prep_tail(0, ci)
        pending = []
        for b in range(BPC):
            emit_mm1(b)
            for ci in range(4):
                if pending:
                    out_dve(*pending[0])
                if b + 1 < BPC:
                    prep_head(b + 1, ci)
                emit_mm2(b, ci)
                if b + 1 < BPC:
                    prep_tail(b + 1, ci)
                if pending:
                    out_pe(*pending.pop(0))
                pending.append((b, ci))
            if b > 1:
                del state[b - 2]
        while pending:
            b, ci = pending.pop(0)
            out_dve(b, ci)
            out_pe(b, ci)

    _split_multi_waits(nc)
    return nc


def _ln_stats(nc, stats_pool, x, eps_t):
    """mean/rstd over the free dim (H=768) via bn_stats in 256-wide groups."""
    sub = 256
    n_sub = H // sub
    st = stats_pool.tile([P, n_sub, 6], f32, tag="bn_st")
    xg = x.rearrange("p (n s) -> p n s", s=sub)
    for i in range(n_sub):
        nc.vector.bn_stats(out=st[:, i, :], in_=xg[:, i, :])
    mv = stats_pool.tile([P, 2], f32, tag="bn_mv")
    nc.vector.bn_aggr(out=mv, in_=st)
    rstd = stats_pool.tile([P, 1], f32, tag="rstd")
    nc.scalar.activation(out=rstd, in_=mv[:, 1:2],
                         func=mybir.ActivationFunctionType.Sqrt,
                         bias=eps_t, scale=1.0)
    nc.vector.reciprocal(out=rstd, in_=rstd)
    return mv[:, 0:1], rstd


def _split_multi_waits(nc, max_waits=1):
    """walrus codegen in this toolchain accepts at most one sync wait per
    compute instruction; hoist extras onto same-engine NoOps just before."""
    n_nops = 0
    for f in nc.m.functions:
        for blk in f.blocks:
            insts = blk.instructions
            out = []
            changed = False
            for inst in insts:
                si = getattr(inst, "sync_info", None)
                waits = list(si.on_wait) if si is not None and si.on_wait else []
                if len(waits) > max_waits:
                    for w in waits[:-max_waits]:
                        nop = mybir.InstNoOp(
                            name=f"W-split-{n_nops}", ins=[], outs=[])
                        nop.engine = inst.engine
                        nop.sync_info = mybir.SyncInfo(on_wait=[w], on_update=[])
                        out.append(nop)
                        n_nops += 1
                    inst.sync_info = mybir.SyncInfo(
                        on_wait=waits[-max_waits:], on_update=list(si.on_update))
                    changed = True
                out.append(inst)
            if changed:
                blk.instructions = out
    return n_nops


_BUILT = {}


def _prep_inputs(word_embedding, tag_emb, w1, b1, w2, b2, g_att, be_att,
                 g_ff, be_ff, w_out, b_out, span_b, span_tag, span_start,
                 span_end):
    """Host-side sharding: bucket spans by batch, cast weights, build in_maps."""
    we = np.ascontiguousarray(np.asarray(word_embedding, np.float32))
    sb = np.asarray(span_b).astype(np.int64)
    stg = np.asarray(span_tag).astype(np.int64)
    ss = np.asarray(span_start).astype(np.int64)
    se = np.asarray(span_end).astype(np.int64)

    counts = np.bincount(sb, minlength=B)
    nt_span = max(1, math.ceil(counts.max() / P))
    smax = nt_span * P
    spans = np.zeros((B, smax, 3), np.float32)
    spans[:, :, 2] = -1.0  # tag -1 never matches iota_t
    for b in range(B):
        idx = np.flatnonzero(sb == b)
        n = len(idx)
        spans[b, :n, 0] = ss[idx]
        spans[b, :n, 1] = se[idx]
        spans[b, :n, 2] = stg[idx]

    w1h = np.asarray(w1, np.float32).astype(np.float16)
    w2h = np.asarray(w2, np.float32).astype(np.float16)
    wouth = np.asarray(w_out, np.float32).astype(np.float16)
    tembh = (np.asarray(tag_emb, np.float32) * RATE).astype(np.float16)

    b1_ = np.asarray(b1, np.float32)
    b2_ = np.asarray(b2, np.float32)
    bout_ = np.asarray(b_out, np.float32)
    ga = np.asarray(g_att, np.float32)
    ba = np.asarray(be_att, np.float32)
    gf = np.asarray(g_ff, np.float32)
    bf = np.asarray(be_ff, np.float32)
    use_b1 = bool(np.any(b1_ != 0))
    use_b2 = bool(np.any(b2_ != 0))
    use_bout = bool(np.any(bout_ != 0))
    use_gb_att = bool(np.any(ga != 1) or np.any(ba != 0))
    use_gb_ff = bool(np.any(gf != 1) or np.any(bf != 0))

    in_maps = []
    for c in range(N_CORES):
        b0 = c * BPC
        m = dict(
            we=we[b0:b0 + BPC].reshape(TOK, H),
            w1=w1h, w2=w2h, wout=wouth, temb=tembh,
            spans=spans[b0:b0 + BPC].reshape(BPC, nt_span, P, 3),
        )
        if use_b1:
            m["b1"] = b1_
        if use_b2:
            m["b2"] = b2_
        if use_bout:
            m["bout"] = bout_
        if use_gb_att:
            m["gb_att"] = np.stack([ga, ba])
        if use_gb_ff:
            m["gb_ff"] = np.stack([gf, bf])
        in_maps.append(m)

    key = (nt_span, use_b1, use_b2, use_bout, use_gb_att, use_gb_ff)
    return key, in_maps


def kernel(**inputs):
    key, in_maps = _prep_inputs(**inputs)
    if key not in _BUILT:
        _BUILT[key] = build_kernel(*key)
    nc = _BUILT[key]
    res = run_bass_kernel_spmd(nc, in_maps, core_ids=list(range(N_CORES)))
    outs = [res.results[c]["out"].reshape(BPC, L, NL) for c in range(N_CORES)]
    return np.concatenate(outs, axis=0).astype(np.float32)


# revision 15
# speedup vs baseline: 1.6396x; 1.0031x over previous
"""Trainium2 Bass kernel for nn_Estor_raw_45595372814583.

Reference computation (B=64, L=512, H=768, I=3072, T=50, NL=9, S=4096):
    taged[b, s:e, :] += tag_emb[tag]      for each span (b, tag, s, e)
    x   = LN(word_embedding + 0.5 * taged) * g_att + be_att
    h   = relu(x @ w1 + b1) @ w2 + b2 + x
    h   = LN(h) * g_ff + be_ff
    out = h @ w_out + b_out               # [B, L, 9]

Strategy: data-parallel over batch across 8 cores (8 batches each). The
span scatter is computed on-device as two small matmuls per batch:
    in_span[s, l] = (l >= start_s) & (l < end_s)        (DVE compares vs iota)
    onehot[s, t]  = (tag_s == t)
    countT[t, l]  = onehot.T @ in_span                  (PE)
    taged[l, :]   = countT[:, l].T @ (0.5 * tag_emb)    (PE)
The FFN runs in fp16 on the PE with fp32 PSUM accumulation; LayerNorm
stats use bn_stats/bn_aggr on the DVE in fp32. Activation transposes go
through the (otherwise idle) DMA XBAR. Batches are software-pipelined:
while batch b runs its second FFN matmul, batch b+1's scatter+LN1 chain
executes on the DVE, and each chunk's output stage is delayed by one
chunk so its LN2 latency hides under the next chunk's matmuls.
"""

import math
import os
import sys

import numpy as np

for _p in ("/opt/trn_rl_repo", "/opt/trn_rl_repo/concourse"):
    if _p not in sys.path and os.path.isdir(_p):
        sys.path.insert(0, _p)

import concourse.bass as bass
import concourse.mybir as mybir
import concourse.tile as tile
from concourse.bass_utils import run_bass_kernel_spmd
from concourse.masks import make_identity

B, L, H, I, T, NL = 64, 512, 768, 3072, 50, 9
RATE = 0.5
EPS = 1e-12
P = 128
N_CORES = 8
BPC = B // N_CORES          # batches per core
TOK = BPC * L               # tokens per core
KSUB = H // P               # 6   k-subtiles over H
ISUB = I // P               # 24  subtiles over I
NH = H // 2                 # 384 n-half for H-wide psum outputs

f32 = mybir.dt.float32
f16 = mybir.dt.float16


def build_kernel(nt_span: int, use_b1: bool, use_b2: bool, use_bout: bool,
                 use_gb_att: bool, use_gb_ff: bool):
    """Build the SPMD Bass program (same program on all 8 cores).

    nt_span: number of 128-span tiles per batch (spans padded to nt_span*128).
    """
    nc = bass.Bass()

    we = nc.declare_dram_parameter("we", [TOK, H], f32, isOutput=False)
    w1 = nc.declare_dram_parameter("w1", [H, I], f16, isOutput=False)
    w2 = nc.declare_dram_parameter("w2", [I, H], f16, isOutput=False)
    wout = nc.declare_dram_parameter("wout", [H, NL], f16, isOutput=False)
    temb = nc.declare_dram_parameter("temb", [T, H], f16, isOutput=False)
    spans = nc.declare_dram_parameter("spans", [BPC, nt_span, P, 3], f32, isOutput=False)
    b1 = nc.declare_dram_parameter("b1", [I], f32, isOutput=False) if use_b1 else None
    b2 = nc.declare_dram_parameter("b2", [H], f32, isOutput=False) if use_b2 else None
    bout = nc.declare_dram_parameter("bout", [NL], f32, isOutput=False) if use_bout else None
    gb_att = nc.declare_dram_parameter("gb_att", [2, H], f32, isOutput=False) if use_gb_att else None
    gb_ff = nc.declare_dram_parameter("gb_ff", [2, H], f32, isOutput=False) if use_gb_ff else None

    iota_l = nc.declare_dram_parameter("iota_l", [L], f32, isOutput=False)
    iota_t = nc.declare_dram_parameter("iota_t", [P], f32, isOutput=False)
    out = nc.declare_dram_parameter("out", [TOK, NL], f32, isOutput=True)

    from contextlib import ExitStack
    with tile.TileContext(nc) as tc, ExitStack() as ctx:
        const = ctx.enter_context(tc.tile_pool(name="const", bufs=1))
        wpool = ctx.enter_context(tc.tile_pool(name="weights", bufs=1))
        span_sb = ctx.enter_context(tc.tile_pool(name="span_sb", bufs=2))
        masks = ctx.enter_context(tc.tile_pool(name="masks", bufs=2))
        chunks = ctx.enter_context(tc.tile_pool(name="chunks", bufs=3))
        mega = ctx.enter_context(tc.tile_pool(name="mega", bufs=2))
        xtp = ctx.enter_context(tc.tile_pool(name="xtp", bufs=2))
        h1pool = ctx.enter_context(tc.tile_pool(name="h1pool", bufs=1))
        ctpool = ctx.enter_context(tc.tile_pool(name="ctpool", bufs=2))
        stats = ctx.enter_context(tc.tile_pool(name="stats", bufs=4))
        outsb = ctx.enter_context(tc.tile_pool(name="outsb", bufs=3))

        # PSUM budget (8 banks): aux 4 + ps2 2 + ps1 2
        pp_aux = ctx.enter_context(tc.tile_pool(name="pp_aux", bufs=4, space="PSUM"))
        pp_mm1 = ctx.enter_context(tc.tile_pool(name="pp_mm1", bufs=2, space="PSUM"))
        pp_mm2 = ctx.enter_context(tc.tile_pool(name="pp_mm2", bufs=2, space="PSUM"))

        # ---- persistent constants / weights ----
        ident = const.tile([P, P], f16)
        make_identity(nc, ident)
        eps_t = const.tile([P, 1], f32)
        nc.vector.memset(eps_t, EPS)
        iota_l_sb = const.tile([P, L], f32)
        nc.sync.dma_start(out=iota_l_sb, in_=iota_l[None, :].to_broadcast([P, L]))
        iota_t_sb = const.tile([P, P], f32)
        nc.sync.dma_start(out=iota_t_sb, in_=iota_t[None, :].to_broadcast([P, P]))

        temb_sb = wpool.tile([P, H], f16)
        if T < P:
            nc.vector.memset(temb_sb, 0.0)
        nc.sync.dma_start(out=temb_sb[:T, :], in_=temb[:, :])
        wout_sb = wpool.tile([P, KSUB, NL], f16)
        nc.sync.dma_start(out=wout_sb, in_=wout.rearrange("(s p) n -> p s n", p=P))
        w1_sb = wpool.tile([P, KSUB, I], f16)
        nc.sync.dma_start(out=w1_sb, in_=w1.rearrange("(s p) i -> p s i", p=P))
        w2_sb = wpool.tile([P, ISUB, H], f16)
        nc.scalar.dma_start(out=w2_sb, in_=w2.rearrange("(s p) h -> p s h", p=P))

        b1_sb = None
        if b1 is not None:
            b1_sb = wpool.tile([P, ISUB], f32)
            nc.sync.dma_start(out=b1_sb, in_=b1.rearrange("(s p) -> p s", p=P))
        b2_sb = None
        if b2 is not None:
            b2_sb = wpool.tile([P, H], f32)
            nc.gpsimd.dma_start(out=b2_sb, in_=b2[None, :].to_broadcast([P, H]))
        bout_sb = None
        if bout is not None:
            bout_sb = wpool.tile([P, NL], f32)
            nc.gpsimd.dma_start(out=bout_sb, in_=bout[None, :].to_broadcast([P, NL]))
        gb_att_sb = None
        if gb_att is not None:
            gb_att_sb = wpool.tile([P, 2, H], f32)
            nc.gpsimd.dma_start(out=gb_att_sb, in_=gb_att[None, :, :].to_broadcast([P, 2, H]))
        gb_ff_sb = None
        if gb_ff is not None:
            gb_ff_sb = wpool.tile([P, 2, H], f32)
            nc.gpsimd.dma_start(out=gb_ff_sb, in_=gb_ff[None, :, :].to_broadcast([P, 2, H]))

        # rotating per-batch state (allocated by the prep stage)
        state = {}

        def emit_count(bp):
            """Span masks + count matmul -> countT_sb [128, L] f16 for batch bp."""
            cnt_psum = pp_aux.tile([P, L], f32, tag="aux")
            for st in range(nt_span):
                sp_t = span_sb.tile([P, 3], f32, tag="spans")
                nc.gpsimd.dma_start(out=sp_t, in_=spans[bp, st, :, :])
                s_t, e_t, g_t = sp_t[:, 0:1], sp_t[:, 1:2], sp_t[:, 2:3]
                ge = masks.tile([P, L], f32, tag="ge")
                nc.vector.tensor_tensor(
                    out=ge, in0=iota_l_sb, in1=s_t.to_broadcast([P, L]),
                    op=mybir.AluOpType.is_ge)
                lt = masks.tile([P, L], f32, tag="lt")
                nc.vector.tensor_tensor(
                    out=lt, in0=iota_l_sb, in1=e_t.to_broadcast([P, L]),
                    op=mybir.AluOpType.is_lt)
                in_span = masks.tile([P, L], f16, tag="in_span")
                nc.vector.tensor_tensor(
                    out=in_span, in0=ge, in1=lt, op=mybir.AluOpType.mult)
                onehot = masks.tile([P, P], f16, tag="onehot")
                nc.vector.tensor_tensor(
                    out=onehot, in0=iota_t_sb, in1=g_t.to_broadcast([P, P]),
                    op=mybir.AluOpType.is_equal)
                nc.tensor.matmul(cnt_psum, lhsT=onehot, rhs=in_span,
                                 start=(st == 0), stop=(st == nt_span - 1))
            countT = ctpool.tile([P, L], f16, tag="countT")
            nc.vector.tensor_copy(out=countT, in_=cnt_psum)
            state[bp] = {"countT": countT}

        def prep_head(bp, ci):
            """taged + LN1 chain (PE: 2 small matmuls; rest DVE)."""
            st_b = state[bp]
            if ci == 0:
                st_b["xn_f32"] = mega.tile([P, 4, H], f32, tag="xn_f32", name="xn_f32")
                st_b["xT"] = xtp.tile([P, KSUB, L], f16, tag="xT", name="xT")
            row0 = bp * L + ci * P
            tg_a = pp_aux.tile([P, NH], f32, tag="aux", name="tg_a")
            tg_b = pp_aux.tile([P, NH], f32, tag="aux", name="tg_b")
            csl = st_b["countT"][:, ci * P:(ci + 1) * P]
            nc.tensor.matmul(tg_a, lhsT=csl, rhs=temb_sb[:, :NH],
                             start=True, stop=True)
            nc.tensor.matmul(tg_b, lhsT=csl, rhs=temb_sb[:, NH:],
                             start=True, stop=True)
            we_t = chunks.tile([P, H], f32, tag="we")
            nc.gpsimd.dma_start(out=we_t, in_=we[row0:row0 + P, :])
            xpre = chunks.tile([P, H], f32, tag="xpre")
            nc.vector.tensor_add(out=xpre[:, :NH], in0=we_t[:, :NH], in1=tg_a)
            nc.vector.tensor_add(out=xpre[:, NH:], in0=we_t[:, NH:], in1=tg_b)

            mean, rstd = _ln_stats(nc, stats, xpre, eps_t)
            xn32 = st_b["xn_f32"][:, ci, :]
            nc.vector.tensor_scalar(
                out=xn32, in0=xpre, scalar1=mean, scalar2=rstd,
                op0=mybir.AluOpType.subtract, op1=mybir.AluOpType.mult)
            if gb_att_sb is not None:
                nc.vector.tensor_mul(out=xn32, in0=xn32, in1=gb_att_sb[:, 0, :])
                nc.vector.tensor_add(out=xn32, in0=xn32, in1=gb_att_sb[:, 1, :])
            xn16 = chunks.tile([P, H], f16, tag="xn16")
            nc.vector.tensor_copy(out=xn16, in_=xn32)
            st_b[("xn16", ci)] = xn16

        def prep_tail(bp, ci):
            """PE transposes of xn16 into xT (emitted after a matmul burst
            so the LN1 chain has already finished on the DVE)."""
            st_b = state[bp]
            xn16 = st_b.pop(("xn16", ci))
            tp_x = pp_aux.tile([P, H], f16, tag="aux", name="tp_x")
            for k in range(KSUB):
                nc.tensor.transpose(tp_x[:, k * P:(k + 1) * P],
                                    xn16[:, k * P:(k + 1) * P], ident)
            nc.vector.tensor_copy(
                out=st_b["xT"][:, :, ci * P:(ci + 1) * P],
                in_=tp_x.rearrange("p (k t) -> p k t", t=P))

        def emit_mm1(b):
            st_b = state[b]
            h1T = h1pool.tile([P, ISUB, L], f16, tag="h1T")
            st_b["h1T"] = h1T
            xT = st_b["xT"]
            for isub in range(ISUB):
                ps1 = pp_mm1.tile([P, L], f32, tag="ps1")
                for k in range(KSUB):
                    nc.tensor.matmul(
                        ps1, lhsT=w1_sb[:, k, isub * P:(isub + 1) * P],
                        rhs=xT[:, k, :], start=(k == 0), stop=(k == KSUB - 1))
                if b1_sb is not None:
                    nc.vector.tensor_scalar(
                        out=h1T[:, isub, :], in0=ps1,
                        scalar1=b1_sb[:, isub:isub + 1], scalar2=0.0,
                        op0=mybir.AluOpType.add, op1=mybir.AluOpType.max)
                else:
                    nc.vector.tensor_scalar(
                        out=h1T[:, isub, :], in0=ps1, scalar1=0.0, scalar2=None,
                        op0=mybir.AluOpType.max)
                if isub == 1 and b + 1 < BPC:
                    emit_count(b + 1)

        def emit_mm2(b, ci):
            st_b = state[b]
            h1T = st_b["h1T"]
            ps2a = pp_mm2.tile([P, NH], f32, tag="ps2", name="ps2a")
            for isub in range(ISUB):
                nc.tensor.matmul(ps2a, lhsT=h1T[:, isub, ci * P:(ci + 1) * P],
                                 rhs=w2_sb[:, isub, :NH],
                                 start=(isub == 0), stop=(isub == ISUB - 1))
            ps2b = pp_mm2.tile([P, NH], f32, tag="ps2", name="ps2b")
            for isub in range(ISUB):
                nc.tensor.matmul(ps2b, lhsT=h1T[:, isub, ci * P:(ci + 1) * P],
                                 rhs=w2_sb[:, isub, NH:],
                                 start=(isub == 0), stop=(isub == ISUB - 1))
            st_b[("ps2", ci)] = (ps2a, ps2b)

        def out_dve(b, ci):
            """residual + LN2 on the DVE (no PE work)."""
            st_b = state[b]
            ps2a, ps2b = st_b.pop(("ps2", ci))
            xn32 = st_b["xn_f32"][:, ci, :]
            h2 = chunks.tile([P, H], f32, tag="h2")
            nc.vector.tensor_add(out=h2[:, :NH], in0=ps2a, in1=xn32[:, :NH])
            nc.vector.tensor_add(out=h2[:, NH:], in0=ps2b, in1=xn32[:, NH:])
            if b2_sb is not None:
                nc.vector.tensor_add(out=h2, in0=h2, in1=b2_sb)

            mean2, rstd2 = _ln_stats(nc, stats, h2, eps_t)
            h2n = chunks.tile([P, H], f16, tag="h2n")
            nc.vector.tensor_scalar(
                out=h2n, in0=h2, scalar1=mean2, scalar2=rstd2,
                op0=mybir.AluOpType.subtract, op1=mybir.AluOpType.mult)
            if gb_ff_sb is not None:
                nc.vector.tensor_mul(out=h2n, in0=h2n, in1=gb_ff_sb[:, 0, :])
                nc.vector.tensor_add(out=h2n, in0=h2n, in1=gb_ff_sb[:, 1, :])
            st_b[("h2n", ci)] = h2n

        def out_pe(b, ci):
            """h2n transposes + output projection (emitted after a matmul
            burst so LN2 has already finished on the DVE)."""
            st_b = state[b]
            h2n = st_b.pop(("h2n", ci))
            row0 = b * L + ci * P
            tp_h = pp_aux.tile([P, H], f16, tag="aux", name="tp_h")
            for k in range(KSUB):
                nc.tensor.transpose(tp_h[:, k * P:(k + 1) * P],
                                    h2n[:, k * P:(k + 1) * P], ident)
            h2nT = chunks.tile([P, H], f16, tag="h2nT")
            nc.vector.tensor_copy(out=h2nT, in_=tp_h)
            ps3 = pp_aux.tile([P, NL], f32, tag="aux", name="ps3")
            for k in range(KSUB):
                nc.tensor.matmul(ps3, lhsT=h2nT[:, k * P:(k + 1) * P],
                                 rhs=wout_sb[:, k, :],
                                 start=(k == 0), stop=(k == KSUB - 1))
            o_t = outsb.tile([P, NL], f32, tag="o")
            if bout_sb is not None:
                nc.vector.tensor_add(out=o_t, in0=ps3, in1=bout_sb)
            else:
                nc.vector.tensor_copy(out=o_t, in_=ps3)
            nc.sync.dma_start(out=out[row0:row0 + P, :], in_=o_t)

        # ---- pipelined emission ----
        emit_count(0)
        for ci in range(4):
            prep_head(0, ci)
        for ci in range(4):
            prep_tail(0, ci)
        pending = []
        for b in range(BPC):
            emit_mm1(b)
            for ci in range(4):
                if pending:
                    out_dve(*pending[0])
                if b + 1 < BPC:
                    prep_head(b + 1, ci)
                emit_mm2(b, ci)
                if b + 1 < BPC:
                    prep_tail(b + 1, ci)
                if pending:
                    out_pe(*pending.pop(0))
                pending.append((b, ci))
            if b > 1:
                del state[b - 2]
        while pending:
            b, ci = pending.pop(0)
            out_dve(b, ci)
            out_pe(b, ci)

    _split_multi_waits(nc)
    return nc


def _ln_stats(nc, stats_pool, x, eps_t):
    """mean/rstd over the free dim (H=768) via bn_stats in 256-wide groups."""
    sub = 256
    n_sub = H // sub
    st = stats_pool.tile([P, n_sub, 6], f32, tag="bn_st")
    xg = x.rearrange("p (n s) -> p n s", s=sub)
    for i in range(n_sub):
        nc.vector.bn_stats(out=st[:, i, :], in_=xg[:, i, :])
    mv = stats_pool.tile([P, 2], f32, tag="bn_mv")
    nc.vector.bn_aggr(out=mv, in_=st)
    rstd = stats_pool.tile([P, 1], f32, tag="rstd")
    nc.scalar.activation(out=rstd, in_=mv[:, 1:2],
                         func=mybir.ActivationFunctionType.Sqrt,
                         bias=eps_t, scale=1.0)
    nc.vector.reciprocal(out=rstd, in_=rstd)
    return mv[:, 0:1], rstd


def _split_multi_waits(nc, max_waits=1):
    """walrus codegen in this toolchain accepts at most one sync wait per
    compute instruction; hoist extras onto same-engine NoOps just before."""
    n_nops = 0
    for f in nc.m.functions:
        for blk in f.blocks:
            insts = blk.instructions
            out = []
            changed = False
            for inst in insts:
                si = getattr(inst, "sync_info", None)
                waits = list(si.on_wait) if si is not None and si.on_wait else []
                if len(waits) > max_waits:
                    for w in waits[:-max_waits]:
                        nop = mybir.InstNoOp(
                            name=f"W-split-{n_nops}", ins=[], outs=[])
                        nop.engine = inst.engine
                        nop.sync_info = mybir.SyncInfo(on_wait=[w], on_update=[])
                        out.append(nop)
                        n_nops += 1
                    inst.sync_info = mybir.SyncInfo(
                        on_wait=waits[-max_waits:], on_update=list(si.on_update))
                    changed = True
                out.append(inst)
            if changed:
                blk.instructions = out
    return n_nops


_BUILT = {}


def _prep_inputs(word_embedding, tag_emb, w1, b1, w2, b2, g_att, be_att,
                 g_ff, be_ff, w_out, b_out, span_b, span_tag, span_start,
                 span_end):
    """Host-side sharding: bucket spans by batch, cast weights, build in_maps."""
    we = np.ascontiguousarray(np.asarray(word_embedding, np.float32))
    sb = np.asarray(span_b).astype(np.int64)
    stg = np.asarray(span_tag).astype(np.int64)
    ss = np.asarray(span_start).astype(np.int64)
    se = np.asarray(span_end).astype(np.int64)

    counts = np.bincount(sb, minlength=B)
    nt_span = max(1, math.ceil(counts.max() / P))
    smax = nt_span * P
    spans = np.zeros((B, smax, 3), np.float32)
    spans[:, :, 2] = -1.0  # tag -1 never matches iota_t
    for b in range(B):
        idx = np.flatnonzero(sb == b)
        n = len(idx)
        spans[b, :n, 0] = ss[idx]
        spans[b, :n, 1] = se[idx]
        spans[b, :n, 2] = stg[idx]

    w1h = np.asarray(w1, np.float32).astype(np.float16)
    w2h = np.asarray(w2, np.float32).astype(np.float16)
    wouth = np.asarray(w_out, np.float32).astype(np.float16)
    tembh = (np.asarray(tag_emb, np.float32) * RATE).astype(np.float16)

    b1_ = np.asarray(b1, np.float32)
    b2_ = np.asarray(b2, np.float32)
    bout_ = np.asarray(b_out, np.float32)
    ga = np.asarray(g_att, np.float32)
    ba = np.asarray(be_att, np.float32)
    gf = np.asarray(g_ff, np.float32)
    bf = np.asarray(be_ff, np.float32)
    use_b1 = bool(np.any(b1_ != 0))
    use_b2 = bool(np.any(b2_ != 0))
    use_bout = bool(np.any(bout_ != 0))
    use_gb_att = bool(np.any(ga != 1) or np.any(ba != 0))
    use_gb_ff = bool(np.any(gf != 1) or np.any(bf != 0))

    in_maps = []
    for c in range(N_CORES):
        b0 = c * BPC
        m = dict(
            we=we[b0:b0 + BPC].reshape(TOK, H),
            w1=w1h, w2=w2h, wout=wouth, temb=tembh,
            spans=spans[b0:b0 + BPC].reshape(BPC, nt_span, P, 3),
            iota_l=np.arange(L, dtype=np.float32),
            iota_t=np.arange(P, dtype=np.float32),
        )
        if use_b1:
            m["b1"] = b1_
        if use_b2:
            m["b2"] = b2_
        if use_bout:
            m["bout"] = bout_
        if use_gb_att:
            m["gb_att"] = np.stack([ga, ba])
        if use_gb_ff:
            m["gb_ff"] = np.stack([gf, bf])
        in_maps.append(m)

    key = (nt_span, use_b1, use_b2, use_bout, use_gb_att, use_gb_ff)
    return key, in_maps


def kernel(**inputs):
    key, in_maps = _prep_inputs(**inputs)
    if key not in _BUILT:
        _BUILT[key] = build_kernel(*key)
    nc = _BUILT[key]
    res = run_bass_kernel_spmd(nc, in_maps, core_ids=list(range(N_CORES)))
    outs = [res.results[c]["out"].reshape(BPC, L, NL) for c in range(N_CORES)]
    return np.concatenate(outs, axis=0).astype(np.float32)


# revision 16
# speedup vs baseline: 1.6566x; 1.0104x over previous
"""Trainium2 Bass kernel for nn_Estor_raw_45595372814583.

Reference computation (B=64, L=512, H=768, I=3072, T=50, NL=9, S=4096):
    taged[b, s:e, :] += tag_emb[tag]      for each span (b, tag, s, e)
    x   = LN(word_embedding + 0.5 * taged) * g_att + be_att
    h   = relu(x @ w1 + b1) @ w2 + b2 + x
    h   = LN(h) * g_ff + be_ff
    out = h @ w_out + b_out               # [B, L, 9]

Strategy: data-parallel over batch across 8 cores (8 batches each). The
span scatter is computed on-device as two small matmuls per batch:
    in_span[s, l] = (l >= start_s) & (l < end_s)        (DVE compares vs iota)
    onehot[s, t]  = (tag_s == t)
    countT[t, l]  = onehot.T @ in_span                  (PE)
    taged[l, :]   = countT[:, l].T @ (0.5 * tag_emb)    (PE)
The FFN runs in fp16 on the PE with fp32 PSUM accumulation; LayerNorm
stats use bn_stats/bn_aggr on the DVE in fp32. Activation transposes go
through the (otherwise idle) DMA XBAR. Batches are software-pipelined:
while batch b runs its second FFN matmul, batch b+1's scatter+LN1 chain
executes on the DVE, and each chunk's output stage is delayed by one
chunk so its LN2 latency hides under the next chunk's matmuls.
"""

import math
import os
import sys

import numpy as np

for _p in ("/opt/trn_rl_repo", "/opt/trn_rl_repo/concourse"):
    if _p not in sys.path and os.path.isdir(_p):
        sys.path.insert(0, _p)

import concourse.bass as bass
import concourse.mybir as mybir
import concourse.tile as tile
from concourse.bass_utils import run_bass_kernel_spmd
from concourse.masks import make_identity

B, L, H, I, T, NL = 64, 512, 768, 3072, 50, 9
RATE = 0.5
EPS = 1e-12
P = 128
N_CORES = 8
BPC = B // N_CORES          # batches per core
TOK = BPC * L               # tokens per core
KSUB = H // P               # 6   k-subtiles over H
ISUB = I // P               # 24  subtiles over I
NH = H // 2                 # 384 n-half for H-wide psum outputs

f32 = mybir.dt.float32
f16 = mybir.dt.float16


def build_kernel(nt_span: int, use_b1: bool, use_b2: bool, use_bout: bool,
                 use_gb_att: bool, use_gb_ff: bool):
    """Build the SPMD Bass program (same program on all 8 cores).

    nt_span: number of 128-span tiles per batch (spans padded to nt_span*128).
    """
    nc = bass.Bass()

    we = nc.declare_dram_parameter("we", [TOK, H], f32, isOutput=False)
    w1 = nc.declare_dram_parameter("w1", [H, I], f16, isOutput=False)
    w2 = nc.declare_dram_parameter("w2", [I, H], f16, isOutput=False)
    wout = nc.declare_dram_parameter("wout", [H, NL], f16, isOutput=False)
    temb = nc.declare_dram_parameter("temb", [T, H], f16, isOutput=False)
    spans = nc.declare_dram_parameter("spans", [BPC, nt_span, P, 3], f32, isOutput=False)
    b1 = nc.declare_dram_parameter("b1", [I], f32, isOutput=False) if use_b1 else None
    b2 = nc.declare_dram_parameter("b2", [H], f32, isOutput=False) if use_b2 else None
    bout = nc.declare_dram_parameter("bout", [NL], f32, isOutput=False) if use_bout else None
    gb_att = nc.declare_dram_parameter("gb_att", [2, H], f32, isOutput=False) if use_gb_att else None
    gb_ff = nc.declare_dram_parameter("gb_ff", [2, H], f32, isOutput=False) if use_gb_ff else None

    iota_l = nc.declare_dram_parameter("iota_l", [L], f32, isOutput=False)
    out = nc.declare_dram_parameter("out", [TOK, NL], f32, isOutput=True)

    from contextlib import ExitStack
    with tile.TileContext(nc) as tc, ExitStack() as ctx:
        const = ctx.enter_context(tc.tile_pool(name="const", bufs=1))
        wpool = ctx.enter_context(tc.tile_pool(name="weights", bufs=1))
        span_sb = ctx.enter_context(tc.tile_pool(name="span_sb", bufs=2))
        masks = ctx.enter_context(tc.tile_pool(name="masks", bufs=2))
        chunks = ctx.enter_context(tc.tile_pool(name="chunks", bufs=3))
        mega = ctx.enter_context(tc.tile_pool(name="mega", bufs=2))
        xtp = ctx.enter_context(tc.tile_pool(name="xtp", bufs=2))
        h1pool = ctx.enter_context(tc.tile_pool(name="h1pool", bufs=1))
        ctpool = ctx.enter_context(tc.tile_pool(name="ctpool", bufs=2))
        stats = ctx.enter_context(tc.tile_pool(name="stats", bufs=4))
        outsb = ctx.enter_context(tc.tile_pool(name="outsb", bufs=3))

        # PSUM budget (8 banks): aux 4 + ps2 2 + ps1 2
        pp_aux = ctx.enter_context(tc.tile_pool(name="pp_aux", bufs=4, space="PSUM"))
        pp_mm1 = ctx.enter_context(tc.tile_pool(name="pp_mm1", bufs=2, space="PSUM"))
        pp_mm2 = ctx.enter_context(tc.tile_pool(name="pp_mm2", bufs=2, space="PSUM"))

        # ---- persistent constants / weights ----
        ident = const.tile([P, P], f16)
        make_identity(nc, ident)
        eps_t = const.tile([P, 1], f32)
        nc.vector.memset(eps_t, EPS)
        iota_row = const.tile([1, L], f32)
        nc.sync.dma_start(out=iota_row, in_=iota_l[None, :])
        ones_col = const.tile([1, P], f32)
        nc.vector.memset(ones_col, 1.0)
        iota_psum = pp_aux.tile([P, L], f32, tag="aux", name="iota_psum")
        nc.tensor.matmul(iota_psum, lhsT=ones_col, rhs=iota_row,
                         start=True, stop=True)
        iota_l_sb = const.tile([P, L], f32)
        nc.vector.tensor_copy(out=iota_l_sb, in_=iota_psum)
        iota_t_sb = iota_l_sb[:, :P]

        temb_sb = wpool.tile([P, H], f16)
        if T < P:
            nc.vector.memset(temb_sb, 0.0)
        nc.sync.dma_start(out=temb_sb[:T, :], in_=temb[:, :])
        wout_sb = wpool.tile([P, KSUB, NL], f16)
        nc.sync.dma_start(out=wout_sb, in_=wout.rearrange("(s p) n -> p s n", p=P))
        w1_sb = wpool.tile([P, KSUB, I], f16)
        nc.sync.dma_start(out=w1_sb, in_=w1.rearrange("(s p) i -> p s i", p=P))
        w2_sb = wpool.tile([P, ISUB, H], f16)
        nc.scalar.dma_start(out=w2_sb, in_=w2.rearrange("(s p) h -> p s h", p=P))

        b1_sb = None
        if b1 is not None:
            b1_sb = wpool.tile([P, ISUB], f32)
            nc.sync.dma_start(out=b1_sb, in_=b1.rearrange("(s p) -> p s", p=P))
        b2_sb = None
        if b2 is not None:
            b2_sb = wpool.tile([P, H], f32)
            nc.gpsimd.dma_start(out=b2_sb, in_=b2[None, :].to_broadcast([P, H]))
        bout_sb = None
        if bout is not None:
            bout_sb = wpool.tile([P, NL], f32)
            nc.gpsimd.dma_start(out=bout_sb, in_=bout[None, :].to_broadcast([P, NL]))
        gb_att_sb = None
        if gb_att is not None:
            gb_att_sb = wpool.tile([P, 2, H], f32)
            nc.gpsimd.dma_start(out=gb_att_sb, in_=gb_att[None, :, :].to_broadcast([P, 2, H]))
        gb_ff_sb = None
        if gb_ff is not None:
            gb_ff_sb = wpool.tile([P, 2, H], f32)
            nc.gpsimd.dma_start(out=gb_ff_sb, in_=gb_ff[None, :, :].to_broadcast([P, 2, H]))

        # rotating per-batch state (allocated by the prep stage)
        state = {}

        def emit_count(bp):
            """Span masks + count matmul -> countT_sb [128, L] f16 for batch bp."""
            cnt_psum = pp_aux.tile([P, L], f32, tag="aux")
            for st in range(nt_span):
                sp_t = span_sb.tile([P, 3], f32, tag="spans")
                nc.gpsimd.dma_start(out=sp_t, in_=spans[bp, st, :, :])
                s_t, e_t, g_t = sp_t[:, 0:1], sp_t[:, 1:2], sp_t[:, 2:3]
                ge = masks.tile([P, L], f32, tag="ge")
                nc.vector.tensor_tensor(
                    out=ge, in0=iota_l_sb, in1=s_t.to_broadcast([P, L]),
                    op=mybir.AluOpType.is_ge)
                lt = masks.tile([P, L], f32, tag="lt")
                nc.vector.tensor_tensor(
                    out=lt, in0=iota_l_sb, in1=e_t.to_broadcast([P, L]),
                    op=mybir.AluOpType.is_lt)
                in_span = masks.tile([P, L], f16, tag="in_span")
                nc.vector.tensor_tensor(
                    out=in_span, in0=ge, in1=lt, op=mybir.AluOpType.mult)
                onehot = masks.tile([P, P], f16, tag="onehot")
                nc.vector.tensor_tensor(
                    out=onehot, in0=iota_t_sb, in1=g_t.to_broadcast([P, P]),
                    op=mybir.AluOpType.is_equal)
                nc.tensor.matmul(cnt_psum, lhsT=onehot, rhs=in_span,
                                 start=(st == 0), stop=(st == nt_span - 1))
            countT = ctpool.tile([P, L], f16, tag="countT")
            nc.vector.tensor_copy(out=countT, in_=cnt_psum)
            state[bp] = {"countT": countT}

        def prep_head(bp, ci):
            """taged + LN1 chain (PE: 2 small matmuls; rest DVE)."""
            st_b = state[bp]
            if ci == 0:
                st_b["xn_f32"] = mega.tile([P, 4, H], f32, tag="xn_f32", name="xn_f32")
                st_b["xT"] = xtp.tile([P, KSUB, L], f16, tag="xT", name="xT")
            row0 = bp * L + ci * P
            tg_a = pp_aux.tile([P, NH], f32, tag="aux", name="tg_a")
            tg_b = pp_aux.tile([P, NH], f32, tag="aux", name="tg_b")
            csl = st_b["countT"][:, ci * P:(ci + 1) * P]
            nc.tensor.matmul(tg_a, lhsT=csl, rhs=temb_sb[:, :NH],
                             start=True, stop=True)
            nc.tensor.matmul(tg_b, lhsT=csl, rhs=temb_sb[:, NH:],
                             start=True, stop=True)
            we_t = chunks.tile([P, H], f32, tag="we")
            nc.gpsimd.dma_start(out=we_t, in_=we[row0:row0 + P, :])
            xpre = chunks.tile([P, H], f32, tag="xpre")
            nc.vector.tensor_add(out=xpre[:, :NH], in0=we_t[:, :NH], in1=tg_a)
            nc.vector.tensor_add(out=xpre[:, NH:], in0=we_t[:, NH:], in1=tg_b)

            mean, rstd = _ln_stats(nc, stats, xpre, eps_t)
            xn32 = st_b["xn_f32"][:, ci, :]
            nc.vector.tensor_scalar(
                out=xn32, in0=xpre, scalar1=mean, scalar2=rstd,
                op0=mybir.AluOpType.subtract, op1=mybir.AluOpType.mult)
            if gb_att_sb is not None:
                nc.vector.tensor_mul(out=xn32, in0=xn32, in1=gb_att_sb[:, 0, :])
                nc.vector.tensor_add(out=xn32, in0=xn32, in1=gb_att_sb[:, 1, :])
            xn16 = chunks.tile([P, H], f16, tag="xn16")
            nc.vector.tensor_copy(out=xn16, in_=xn32)
            st_b[("xn16", ci)] = xn16

        def prep_tail(bp, ci):
            """PE transposes of xn16 into xT (emitted after a matmul burst
            so the LN1 chain has already finished on the DVE)."""
            st_b = state[bp]
            xn16 = st_b.pop(("xn16", ci))
            tp_x = pp_aux.tile([P, H], f16, tag="aux", name="tp_x")
            for k in range(KSUB):
                nc.tensor.transpose(tp_x[:, k * P:(k + 1) * P],
                                    xn16[:, k * P:(k + 1) * P], ident)
            nc.vector.tensor_copy(
                out=st_b["xT"][:, :, ci * P:(ci + 1) * P],
                in_=tp_x.rearrange("p (k t) -> p k t", t=P))

        def emit_mm1(b):
            st_b = state[b]
            h1T = h1pool.tile([P, ISUB, L], f16, tag="h1T")
            st_b["h1T"] = h1T
            xT = st_b["xT"]
            for isub in range(ISUB):
                ps1 = pp_mm1.tile([P, L], f32, tag="ps1")
                for k in range(KSUB):
                    nc.tensor.matmul(
                        ps1, lhsT=w1_sb[:, k, isub * P:(isub + 1) * P],
                        rhs=xT[:, k, :], start=(k == 0), stop=(k == KSUB - 1))
                if b1_sb is not None:
                    nc.vector.tensor_scalar(
                        out=h1T[:, isub, :], in0=ps1,
                        scalar1=b1_sb[:, isub:isub + 1], scalar2=0.0,
                        op0=mybir.AluOpType.add, op1=mybir.AluOpType.max)
                else:
                    nc.vector.tensor_scalar(
                        out=h1T[:, isub, :], in0=ps1, scalar1=0.0, scalar2=None,
                        op0=mybir.AluOpType.max)
                if isub == 1 and b + 1 < BPC:
                    emit_count(b + 1)

        def emit_mm2(b, ci):
            st_b = state[b]
            h1T = st_b["h1T"]
            ps2a = pp_mm2.tile([P, NH], f32, tag="ps2", name="ps2a")
            for isub in range(ISUB):
                nc.tensor.matmul(ps2a, lhsT=h1T[:, isub, ci * P:(ci + 1) * P],
                                 rhs=w2_sb[:, isub, :NH],
                                 start=(isub == 0), stop=(isub == ISUB - 1))
            ps2b = pp_mm2.tile([P, NH], f32, tag="ps2", name="ps2b")
            for isub in range(ISUB):
                nc.tensor.matmul(ps2b, lhsT=h1T[:, isub, ci * P:(ci + 1) * P],
                                 rhs=w2_sb[:, isub, NH:],
                                 start=(isub == 0), stop=(isub == ISUB - 1))
            st_b[("ps2", ci)] = (ps2a, ps2b)

        def out_dve(b, ci):
            """residual + LN2 on the DVE (no PE work)."""
            st_b = state[b]
            ps2a, ps2b = st_b.pop(("ps2", ci))
            xn32 = st_b["xn_f32"][:, ci, :]
            h2 = chunks.tile([P, H], f32, tag="h2")
            nc.vector.tensor_add(out=h2[:, :NH], in0=ps2a, in1=xn32[:, :NH])
            nc.vector.tensor_add(out=h2[:, NH:], in0=ps2b, in1=xn32[:, NH:])
            if b2_sb is not None:
                nc.vector.tensor_add(out=h2, in0=h2, in1=b2_sb)

            mean2, rstd2 = _ln_stats(nc, stats, h2, eps_t)
            h2n = chunks.tile([P, H], f16, tag="h2n")
            nc.vector.tensor_scalar(
                out=h2n, in0=h2, scalar1=mean2, scalar2=rstd2,
                op0=mybir.AluOpType.subtract, op1=mybir.AluOpType.mult)
            if gb_ff_sb is not None:
                nc.vector.tensor_mul(out=h2n, in0=h2n, in1=gb_ff_sb[:, 0, :])
                nc.vector.tensor_add(out=h2n, in0=h2n, in1=gb_ff_sb[:, 1, :])
            st_b[("h2n", ci)] = h2n

        def out_pe(b, ci):
            """h2n transposes + output projection (emitted after a matmul
            burst so LN2 has already finished on the DVE)."""
            st_b = state[b]
            h2n = st_b.pop(("h2n", ci))
            row0 = b * L + ci * P
            tp_h = pp_aux.tile([P, H], f16, tag="aux", name="tp_h")
            for k in range(KSUB):
                nc.tensor.transpose(tp_h[:, k * P:(k + 1) * P],
                                    h2n[:, k * P:(k + 1) * P], ident)
            h2nT = chunks.tile([P, H], f16, tag="h2nT")
            nc.vector.tensor_copy(out=h2nT, in_=tp_h)
            ps3 = pp_aux.tile([P, NL], f32, tag="aux", name="ps3")
            for k in range(KSUB):
                nc.tensor.matmul(ps3, lhsT=h2nT[:, k * P:(k + 1) * P],
                                 rhs=wout_sb[:, k, :],
                                 start=(k == 0), stop=(k == KSUB - 1))
            o_t = outsb.tile([P, NL], f32, tag="o")
            if bout_sb is not None:
                nc.vector.tensor_add(out=o_t, in0=ps3, in1=bout_sb)
            else:
                nc.vector.tensor_copy(out=o_t, in_=ps3)
            nc.sync.dma_start(out=out[row0:row0 + P, :], in_=o_t)

        # ---- pipelined emission ----
        emit_count(0)
        for ci in range(4):
            prep_head(0, ci)
        for ci in range(4):
            prep_tail(0, ci)
        pending = []
        for b in range(BPC):
            emit_mm1(b)
            for ci in range(4):
                if pending:
                    out_dve(*pending[0])
                if b + 1 < BPC:
                    prep_head(b + 1, ci)
                emit_mm2(b, ci)
                if b + 1 < BPC:
                    prep_tail(b + 1, ci)
                if pending:
                    out_pe(*pending.pop(0))
                pending.append((b, ci))
            if b > 1:
                del state[b - 2]
        while pending:
            b, ci = pending.pop(0)
            out_dve(b, ci)
            out_pe(b, ci)

    _split_multi_waits(nc)
    return nc


def _ln_stats(nc, stats_pool, x, eps_t):
    """mean/rstd over the free dim (H=768) via bn_stats in 256-wide groups."""
    sub = 256
    n_sub = H // sub
    st = stats_pool.tile([P, n_sub, 6], f32, tag="bn_st")
    xg = x.rearrange("p (n s) -> p n s", s=sub)
    for i in range(n_sub):
        nc.vector.bn_stats(out=st[:, i, :], in_=xg[:, i, :])
    mv = stats_pool.tile([P, 2], f32, tag="bn_mv")
    nc.vector.bn_aggr(out=mv, in_=st)
    rstd = stats_pool.tile([P, 1], f32, tag="rstd")
    nc.scalar.activation(out=rstd, in_=mv[:, 1:2],
                         func=mybir.ActivationFunctionType.Sqrt,
                         bias=eps_t, scale=1.0)
    nc.vector.reciprocal(out=rstd, in_=rstd)
    return mv[:, 0:1], rstd


def _split_multi_waits(nc, max_waits=1):
    """walrus codegen in this toolchain accepts at most one sync wait per
    compute instruction; hoist extras onto same-engine NoOps just before."""
    n_nops = 0
    for f in nc.m.functions:
        for blk in f.blocks:
            insts = blk.instructions
            out = []
            changed = False
            for inst in insts:
                si = getattr(inst, "sync_info", None)
                waits = list(si.on_wait) if si is not None and si.on_wait else []
                if len(waits) > max_waits:
                    for w in waits[:-max_waits]:
                        nop = mybir.InstNoOp(
                            name=f"W-split-{n_nops}", ins=[], outs=[])
                        nop.engine = inst.engine
                        nop.sync_info = mybir.SyncInfo(on_wait=[w], on_update=[])
                        out.append(nop)
                        n_nops += 1
                    inst.sync_info = mybir.SyncInfo(
                        on_wait=waits[-max_waits:], on_update=list(si.on_update))
                    changed = True
                out.append(inst)
            if changed:
                blk.instructions = out
    return n_nops


_BUILT = {}


def _prep_inputs(word_embedding, tag_emb, w1, b1, w2, b2, g_att, be_att,
                 g_ff, be_ff, w_out, b_out, span_b, span_tag, span_start,
                 span_end):
    """Host-side sharding: bucket spans by batch, cast weights, build in_maps."""
    we = np.ascontiguousarray(np.asarray(word_embedding, np.float32))
    sb = np.asarray(span_b).astype(np.int64)
    stg = np.asarray(span_tag).astype(np.int64)
    ss = np.asarray(span_start).astype(np.int64)
    se = np.asarray(span_end).astype(np.int64)

    counts = np.bincount(sb, minlength=B)
    nt_span = max(1, math.ceil(counts.max() / P))
    smax = nt_span * P
    spans = np.zeros((B, smax, 3), np.float32)
    spans[:, :, 2] = -1.0  # tag -1 never matches iota_t
    for b in range(B):
        idx = np.flatnonzero(sb == b)
        n = len(idx)
        spans[b, :n, 0] = ss[idx]
        spans[b, :n, 1] = se[idx]
        spans[b, :n, 2] = stg[idx]

    w1h = np.asarray(w1, np.float32).astype(np.float16)
    w2h = np.asarray(w2, np.float32).astype(np.float16)
    wouth = np.asarray(w_out, np.float32).astype(np.float16)
    tembh = (np.asarray(tag_emb, np.float32) * RATE).astype(np.float16)

    b1_ = np.asarray(b1, np.float32)
    b2_ = np.asarray(b2, np.float32)
    bout_ = np.asarray(b_out, np.float32)
    ga = np.asarray(g_att, np.float32)
    ba = np.asarray(be_att, np.float32)
    gf = np.asarray(g_ff, np.float32)
    bf = np.asarray(be_ff, np.float32)
    use_b1 = bool(np.any(b1_ != 0))
    use_b2 = bool(np.any(b2_ != 0))
    use_bout = bool(np.any(bout_ != 0))
    use_gb_att = bool(np.any(ga != 1) or np.any(ba != 0))
    use_gb_ff = bool(np.any(gf != 1) or np.any(bf != 0))

    in_maps = []
    for c in range(N_CORES):
        b0 = c * BPC
        m = dict(
            we=we[b0:b0 + BPC].reshape(TOK, H),
            w1=w1h, w2=w2h, wout=wouth, temb=tembh,
            spans=spans[b0:b0 + BPC].reshape(BPC, nt_span, P, 3),
            iota_l=np.arange(L, dtype=np.float32),
        )
        if use_b1:
            m["b1"] = b1_
        if use_b2:
            m["b2"] = b2_
        if use_bout:
            m["bout"] = bout_
        if use_gb_att:
            m["gb_att"] = np.stack([ga, ba])
        if use_gb_ff:
            m["gb_ff"] = np.stack([gf, bf])
        in_maps.append(m)

    key = (nt_span, use_b1, use_b2, use_bout, use_gb_att, use_gb_ff)
    return key, in_maps


def kernel(**inputs):
    key, in_maps = _prep_inputs(**inputs)
    if key not in _BUILT:
        _BUILT[key] = build_kernel(*key)
    nc = _BUILT[key]
    res = run_bass_kernel_spmd(nc, in_maps, core_ids=list(range(N_CORES)))
    outs = [res.results[c]["out"].reshape(BPC, L, NL) for c in range(N_CORES)]
    return np.concatenate(outs, axis=0).astype(np.float32)
